# revision 1
# baseline (speedup 1.0000x reference)
"""GATv2 (2-layer) + pooling + LayerNorm + MLP encoder — Trainium2 Bass kernel.

Sharding: data-parallel over graphs — 8 graphs, one per NeuronCore. All
conv/pool/FC weights replicated. Inside each core everything runs per-graph
(2000 nodes, 64000 edges + 2000 self-loops).

Node slot space: 16 chunks x 128 slots; real node n -> slot 128*(n//125) +
(n%125), so every chunk has 3 spare slots used as scatter targets for edge
padding. Edges are sorted by destination chunk and padded per chunk to a
uniform SPC slots (128 edges each) so one SPMD program serves all graphs.

Per edge-slot (128 edges across partitions, features along free):
  gather xl[src], xr[dst] rows via gpsimd.dma_gather (DRAM bf16 tables),
  v = xl_s + xr_d; u = leaky_relu(v); t = u * att; logits = reduce_C(t);
  exp -> alpha-numerators; W = xl_s * exp; aggregation + softmax denominator
  via a single PE matmul per slot: onehot(dst)^T @ [W | exp] accumulated in
  PSUM per chunk; epilogue divides by denominator, adds bias, relu.
"""

import sys

sys.path.insert(0, "/opt/trn_rl_repo")

from contextlib import ExitStack

import ml_dtypes
import numpy as np

import concourse.bass as bass
import concourse.bass_isa as bass_isa
import concourse.tile as tile
from concourse import bacc, bass_utils, mybir

BF16 = ml_dtypes.bfloat16
F32 = np.float32
DT = mybir.dt
ALU = mybir.AluOpType
ACT = mybir.ActivationFunctionType

NUM_NODES = 2000
B = 8
IN = 128
NH, C = 4, 32
HC = 128
NEG = 0.2
EPS = 1e-5
FC1, FC2 = 512, 128
P = 128
NCH = 16          # node chunks
CPN = 125         # real nodes per chunk
NPAD = NCH * P    # 2048 slots


def _slot(n):
    return (n // CPN) * P + (n % CPN)


def _ap(a, dims):
    return bass.AP(a.tensor, a.offset, dims)


def _bcast_mid(a, count):
    """[P, F] -> [P, count, F] with step-0 middle dim."""
    return bass.AP(a.tensor, a.offset,
                   [list(a.ap[0]), [0, count], list(a.ap[1])])


def _head_view(a, inner_pitch):
    """[..., 128]-inner AP -> [..., 4, 32] head view."""
    dims = [list(d) for d in a.ap[:-1]] + [[C, NH], [1, C]]
    assert a.ap[-1][1] == HC and a.ap[-1][0] == 1, list(a.ap[-1])
    return bass.AP(a.tensor, a.offset, dims)


def _exp_bcast(a):
    """[..., 4]-inner AP -> [..., 4, 32] with step-0 C dim."""
    dims = [list(d) for d in a.ap[:-1]] + [[1, NH], [0, C]]
    assert a.ap[-1][1] == NH
    return bass.AP(a.tensor, a.offset, dims)


INPUT_SPECS = None  # set below


def _input_specs(SPC):
    NSLOTS = NCH * SPC
    return [
        ("xpad", [NPAD, IN], DT.float32),
        ("ewsrc", [P, NSLOTS * 8], DT.int16),
        ("ewdst", [P, NSLOTS * 8], DT.int16),
        ("dstloc", [P, NSLOTS], DT.float32),
        ("iota", [P, P], DT.bfloat16),
        ("idf", [P, P], DT.float32),
        ("idb", [P, P], DT.bfloat16),
        ("one11", [1, 1], DT.float32),
        ("wl1T", [P, P], DT.float32), ("wr1T", [P, P], DT.float32),
        ("wl2T", [P, P], DT.float32), ("wr2T", [P, P], DT.float32),
        ("bl1b", [P, P], DT.float32), ("br1b", [P, P], DT.float32),
        ("bias1b", [P, P], DT.float32), ("bl2b", [P, P], DT.float32),
        ("br2b", [P, P], DT.float32), ("bias2b", [P, P], DT.float32),
        ("att1b", [P, P], DT.bfloat16), ("att2b", [P, P], DT.bfloat16),
        ("wp1b", [P, P], DT.float32), ("wp2b", [P, P], DT.float32),
        ("lng", [P, NCH], DT.float32),
        ("lnbt", [P, NCH], DT.float32),
        ("maskc", [P, NCH], DT.float32),
        ("we1ts", [P, 48 * FC1], DT.bfloat16),
        ("be1r", [1, FC1], DT.float32),
        ("we2t", [P, 4 * P], DT.float32),
        ("be2r", [1, FC2], DT.float32),
        ("we3t", [P, 1], DT.float32),
        ("be3r", [1, 1], DT.float32),
    ]


def _trace_program(tc, ins, pred_ap, SPC, stop_after=None):
    """ins: dict name -> dram AP; pred_ap: output dram AP."""
    nc = tc.nc
    NSLOTS = NCH * SPC

    class _H:  # minimal handle-like wrapper so .ap() works uniformly
        def __init__(self, ap):
            self._ap = ap

        def ap(self):
            return self._ap

    xpad_d = _H(ins["xpad"])
    ewsrc_d = _H(ins["ewsrc"])
    ewdst_d = _H(ins["ewdst"])
    dstloc_d = _H(ins["dstloc"])
    iota_d = _H(ins["iota"])
    idf_d = _H(ins["idf"])
    idb_d = _H(ins["idb"])
    one11_d = _H(ins["one11"])
    wts = {nm: _H(ins[nm])
           for nm in ["wl1T", "wr1T", "wl2T", "wr2T", "bl1b", "br1b",
                      "bias1b", "bl2b", "br2b", "bias2b", "att1b", "att2b",
                      "wp1b", "wp2b"]}
    lng_d = _H(ins["lng"])
    lnb_d = _H(ins["lnbt"])
    mask_d = _H(ins["maskc"])
    we1_d = _H(ins["we1ts"])
    be1_d = _H(ins["be1r"])
    we2_d = _H(ins["we2t"])
    be2_d = _H(ins["be2r"])
    we3_d = _H(ins["we3t"])
    be3_d = _H(ins["be3r"])
    pred_d = _H(pred_ap)

    # internal DRAM gather tables
    tbl = {nm: nc.dram_tensor(f"{nm}_scr", [NPAD, HC], DT.bfloat16,
                              kind="Internal")
           for nm in ["tl1", "tr1", "tl2", "tr2"]}

    with ExitStack() as ctx:
        pers = ctx.enter_context(tc.tile_pool(name="pers", bufs=1))
        mega = ctx.enter_context(tc.tile_pool(name="mega", bufs=2))
        ohp = ctx.enter_context(tc.tile_pool(name="ohp", bufs=4))
        sm = ctx.enter_context(tc.tile_pool(name="sm", bufs=2))
        psum = ctx.enter_context(tc.tile_pool(name="psum", bufs=2, space="PSUM"))
        psumz = ctx.enter_context(tc.tile_pool(name="psumz", bufs=1, space="PSUM"))

        def load(d, shape, dt, tag):
            t = pers.tile(shape, dt, tag=tag)
            nc.sync.dma_start(t[:], d.ap())
            return t

        # ---- constant loads
        iota_t = load(iota_d, [P, P], DT.bfloat16, "iota")
        idf_t = load(idf_d, [P, P], DT.float32, "idf")
        idb_t = load(idb_d, [P, P], DT.bfloat16, "idb")
        one11_t = load(one11_d, [1, 1], DT.float32, "one11")
        w = {}
        for nm, d in wts.items():
            dt_w = DT.bfloat16 if nm in ("att1b", "att2b") else DT.float32
            w[nm] = load(d, [P, P], dt_w, nm)
        lng_t = load(lng_d, [P, NCH], DT.float32, "lng")
        lnb_t = load(lnb_d, [P, NCH], DT.float32, "lnbt")
        mask_t = load(mask_d, [P, NCH], DT.float32, "maskc")
        we1_t = load(we1_d, [P, 48 * FC1], DT.bfloat16, "we1")
        be1_t = load(be1_d, [1, FC1], DT.float32, "be1")
        we2_t = load(we2_d, [P, 4 * P], DT.float32, "we2")
        be2_t = load(be2_d, [1, FC2], DT.float32, "be2")
        we3_t = load(we3_d, [P, 1], DT.float32, "we3")
        be3_t = load(be3_d, [1, 1], DT.float32, "be3")
        ewsrc_t = load(ewsrc_d, [P, NSLOTS * 8], DT.int16, "ewsrc")
        ewdst_t = load(ewdst_d, [P, NSLOTS * 8], DT.int16, "ewdst")
        dstloc_t = load(dstloc_d, [P, NSLOTS], DT.float32, "dstloc")

        # ---- x load [p, c, f]
        x_t = pers.tile([P, NCH, IN], DT.float32, tag="x")
        nc.sync.dma_start(x_t[:], xpad_d.ap().rearrange("(c p) f -> p c f", p=P))

        # ---- x0 pooling column + xT
        x0col = pers.tile([P, NCH], DT.float32, tag="x0col")
        x1col = pers.tile([P, NCH], DT.float32, tag="x1col")
        x2col = pers.tile([P, NCH], DT.float32, tag="x2col")
        xT_t = pers.tile([P, NCH, IN], DT.float32, tag="xT")
        for c in range(NCH):
            nc.vector.tensor_reduce(x0col[:, c:c + 1], x_t[:, c, :],
                                    axis=mybir.AxisListType.X, op=ALU.add)
            tp = psum.tile([P, P], DT.float32, tag="tp", space="PSUM")
            nc.tensor.transpose(tp[:], x_t[:, c, :], idf_t[:])
            nc.vector.tensor_copy(xT_t[:, c, :], tp[:])
        nc.vector.tensor_scalar(x0col[:], x0col[:], 1.0 / IN, None, op0=ALU.mult)

        def make_tables(lhsT_tile, wl, bl, wr, br, dl, dr):
            for c in range(NCH):
                for wmat, bvec, dst in ((wl, bl, dl), (wr, br, dr)):
                    pm = psum.tile([P, P], DT.float32, tag="tp", space="PSUM")
                    nc.tensor.matmul(pm[:], lhsT_tile[:, c, :], wmat[:],
                                     start=True, stop=True)
                    tb = sm.tile([P, P], DT.bfloat16, tag="tbl")
                    nc.vector.tensor_tensor(tb[:], pm[:], bvec[:], op=ALU.add)
                    nc.sync.dma_start(dst.ap()[c * P:(c + 1) * P, :], tb[:])

        def _finish():
            zf = sm.tile([1, 1], DT.float32, tag="zfin")
            nc.vector.memset(zf[:], 0.0)
            nc.sync.dma_start(pred_d.ap(), zf[:])

        if stop_after == "A0":
            _finish(); return

        make_tables(xT_t, w["wl1T"], w["bl1b"], w["wr1T"], w["br1b"],
                    tbl["tl1"], tbl["tr1"])
        if stop_after == "A":
            _finish(); return

        MSL = 32  # slots per gather mega (4096 idx = SWDGE ring limit)

        def gat_layer(tblL, tblR, att_t, biasO_t, h_out, nch=NCH,
                      skip_gather=False, gather_only=False):
            nslots = nch * SPC
            nmega = (nslots + MSL - 1) // MSL
            wexp_tiles = {}
            agg_tiles = {}
            for m in range(nmega):
                s0 = m * MSL
                ns = min(MSL, nslots - s0)
                nidx = ns * P
                srcg = mega.tile([P, ns, HC], DT.bfloat16, tag="srcg")
                dstg = mega.tile([P, ns, HC], DT.bfloat16, tag="dstg")
                if skip_gather:
                    nc.vector.memset(srcg[:], 0.25)
                    nc.vector.memset(dstg[:], 0.25)
                else:
                    nc.gpsimd.dma_gather(
                        srcg[:], tblL.ap(), ewsrc_t[:, s0 * 8:(s0 + ns) * 8],
                        nidx, nidx, elem_size=HC, queue_num=0,
                        single_packet=False)
                    nc.gpsimd.dma_gather(
                        dstg[:], tblR.ap(), ewdst_t[:, s0 * 8:(s0 + ns) * 8],
                        nidx, nidx, elem_size=HC, queue_num=0,
                        single_packet=False)
                if gather_only:
                    continue
                v = mega.tile([P, ns, HC], DT.bfloat16, tag="v")
                nc.vector.tensor_tensor(v[:], srcg[:], dstg[:], op=ALU.add)
                nc.vector.scalar_tensor_tensor(v[:], v[:], NEG, v[:],
                                               op0=ALU.mult, op1=ALU.max)
                nc.vector.tensor_tensor(v[:], v[:], _bcast_mid(att_t[:], ns),
                                        op=ALU.mult)
                lg = mega.tile([P, ns, NH], DT.float32, tag="lg")
                nc.vector.tensor_reduce(lg[:], _head_view(v[:], HC),
                                        axis=mybir.AxisListType.X, op=ALU.add)
                wexp = mega.tile([P, ns, HC + NH], DT.bfloat16, tag="wexp")
                nc.scalar.activation(wexp[:, :, HC:HC + NH], lg[:], ACT.Exp)
                nc.vector.tensor_tensor(
                    _head_view(wexp[:, :, 0:HC], HC + NH),
                    _head_view(srcg[:], HC),
                    _exp_bcast(wexp[:, :, HC:HC + NH]),
                    op=ALU.mult)
                wexp_tiles[m] = wexp
                # aggregation for the slots in this mega
                for sl in range(ns):
                    s = s0 + sl
                    c = s // SPC
                    if s == c * SPC:
                        agg = psum.tile([P, HC + NH], DT.float32,
                                        tag="agg", space="PSUM")
                        agg_tiles[c] = agg
                    agg = agg_tiles[c]
                    oh = ohp.tile([P, P], DT.bfloat16, tag="oh")
                    nc.vector.tensor_scalar(oh[:], iota_t[:],
                                            dstloc_t[:, s:s + 1], None,
                                            op0=ALU.is_equal)
                    nc.tensor.matmul(agg[:], oh[:], wexp[:, sl, :],
                                     start=(s == c * SPC),
                                     stop=(s == (c + 1) * SPC - 1))
                    if s == (c + 1) * SPC - 1:
                        # epilogue: h = relu(agg / den + bias)
                        den = sm.tile([P, NH], DT.float32, tag="den")
                        nc.vector.tensor_scalar(den[:], agg[:, HC:HC + NH],
                                                1e-6, None, op0=ALU.add)
                        rec = sm.tile([P, NH], DT.float32, tag="rec")
                        nc.vector.reciprocal(rec[:], den[:])
                        t1 = sm.tile([P, P], DT.float32, tag="t1")
                        nc.vector.tensor_tensor(
                            _head_view(t1[:], P),
                            _head_view(agg[:, 0:HC], HC + NH),
                            _exp_bcast(rec[:]), op=ALU.mult)
                        t2 = sm.tile([P, P], DT.float32, tag="t2")
                        nc.vector.tensor_tensor(t2[:], t1[:], biasO_t[:],
                                                op=ALU.add)
                        nc.scalar.activation(h_out[:, c, :], t2[:], ACT.Relu)
                        del agg_tiles[c]

        if stop_after in ("G1", "G16", "NG1", "NG16", "C1"):
            hx = pers.tile([P, NCH, HC], DT.float32, tag="h1")
            n = 1 if stop_after in ("G1", "NG1", "C1") else NCH
            gat_layer(tbl["tl1"], tbl["tr1"], w["att1b"], w["bias1b"], hx,
                      nch=n, skip_gather=stop_after.startswith("NG"),
                      gather_only=stop_after.startswith("G"))
            _finish(); return

        h1_t = pers.tile([P, NCH, HC], DT.float32, tag="h1")
        gat_layer(tbl["tl1"], tbl["tr1"], w["att1b"], w["bias1b"], h1_t)
        if stop_after == "L1":
            _finish(); return

        # pooling x1, h1 transpose, layer-2 tables
        def pool_into(h_tile, wp_t, outcol):
            for c in range(NCH):
                pt = sm.tile([P, P], DT.float32, tag="ptmp")
                nc.vector.tensor_tensor(pt[:], h_tile[:, c, :], wp_t[:],
                                        op=ALU.mult)
                nc.vector.tensor_reduce(outcol[:, c:c + 1], pt[:],
                                        axis=mybir.AxisListType.X, op=ALU.add)

        pool_into(h1_t, w["wp1b"], x1col)
        h1T_t = pers.tile([P, NCH, HC], DT.float32, tag="h1T")
        for c in range(NCH):
            tp = psum.tile([P, P], DT.float32, tag="tp", space="PSUM")
            nc.tensor.transpose(tp[:], h1_t[:, c, :], idf_t[:])
            nc.vector.tensor_copy(h1T_t[:, c, :], tp[:])
        make_tables(h1T_t, w["wl2T"], w["bl2b"], w["wr2T"], w["br2b"],
                    tbl["tl2"], tbl["tr2"])
        if stop_after == "MID":
            _finish(); return

        h2_t = pers.tile([P, NCH, HC], DT.float32, tag="h2")
        gat_layer(tbl["tl2"], tbl["tr2"], w["att2b"], w["bias2b"], h2_t)
        pool_into(h2_t, w["wp2b"], x2col)
        if stop_after == "L2":
            _finish(); return

        # ---- LayerNorm on the three pooled rows -> bf16 columns [P, 48]
        lncols = pers.tile([P, 3 * NCH], DT.float32, tag="lncols")

        def layer_norm(xcol, colbase):
            xm = sm.tile([P, NCH], DT.float32, tag="xm")
            nc.vector.tensor_tensor(xm[:], xcol[:], mask_t[:], op=ALU.mult)
            alr = sm.tile([P, NCH], DT.float32, tag="alr")
            nc.gpsimd.partition_all_reduce(alr[:], xm[:], P,
                                           bass_isa.ReduceOp.add)
            tot = sm.tile([P, 1], DT.float32, tag="tot")
            nc.vector.tensor_reduce(tot[:], alr[:],
                                    axis=mybir.AxisListType.X, op=ALU.add)
            mean = sm.tile([P, 1], DT.float32, tag="mean")
            nc.vector.tensor_scalar(mean[:], tot[:], 1.0 / NUM_NODES, None,
                                    op0=ALU.mult)
            sq = sm.tile([P, NCH], DT.float32, tag="sq")
            nc.vector.tensor_tensor(sq[:], xm[:], xm[:], op=ALU.mult)
            alr2 = sm.tile([P, NCH], DT.float32, tag="alr2")
            nc.gpsimd.partition_all_reduce(alr2[:], sq[:], P,
                                           bass_isa.ReduceOp.add)
            tot2 = sm.tile([P, 1], DT.float32, tag="tot2")
            nc.vector.tensor_reduce(tot2[:], alr2[:],
                                    axis=mybir.AxisListType.X, op=ALU.add)
            msq = sm.tile([P, 1], DT.float32, tag="msq")
            nc.vector.tensor_scalar(msq[:], tot2[:], 1.0 / NUM_NODES, None,
                                    op0=ALU.mult)
            m2 = sm.tile([P, 1], DT.float32, tag="m2")
            nc.vector.tensor_tensor(m2[:], mean[:], mean[:], op=ALU.mult)
            var = sm.tile([P, 1], DT.float32, tag="var")
            nc.vector.tensor_tensor(var[:], msq[:], m2[:], op=ALU.subtract)
            sd = sm.tile([P, 1], DT.float32, tag="sd")
            nc.scalar.activation(sd[:], var[:], ACT.Sqrt, bias=eps_t[:])
            rstd = sm.tile([P, 1], DT.float32, tag="rstd")
            nc.vector.reciprocal(rstd[:], sd[:])
            rg = sm.tile([P, NCH], DT.float32, tag="rg")
            nc.vector.tensor_scalar(rg[:], lng_t[:], rstd[:], None,
                                    op0=ALU.mult)
            lnv = sm.tile([P, NCH], DT.float32, tag="lnv")
            nc.vector.scalar_tensor_tensor(lnv[:], xm[:], mean[:], rg[:],
                                           op0=ALU.subtract, op1=ALU.mult)
            nc.vector.tensor_tensor(lncols[:, colbase:colbase + NCH],
                                    lnv[:], lnb_t[:], op=ALU.add)

        eps_t = pers.tile([P, 1], DT.float32, tag="eps")
        nc.vector.memset(eps_t[:], EPS)

        layer_norm(x0col, 0)
        layer_norm(x1col, NCH)
        layer_norm(x2col, 2 * NCH)
        if stop_after == "LN":
            _finish(); return

        # ---- FC encoder
        lnbf = pers.tile([P, 3 * NCH], DT.bfloat16, tag="lnbf")
        nc.vector.tensor_copy(lnbf[:], lncols[:])
        z1p = psumz.tile([1, FC1], DT.float32, tag="zacc", space="PSUM")
        for k in range(48):
            nc.tensor.matmul(z1p[:], lnbf[:, k:k + 1],
                             we1_t[:, k * FC1:(k + 1) * FC1],
                             start=(k == 0), stop=(k == 47))
        z1s = sm.tile([1, FC1], DT.float32, tag="z1s")
        nc.vector.tensor_tensor(z1s[:], z1p[:], be1_t[:], op=ALU.add)
        z1b = sm.tile([1, FC1], DT.float32, tag="z1b")
        nc.scalar.activation(z1b[:], z1s[:], ACT.Relu)
        z1c = sm.tile([P, 4], DT.float32, tag="z1c")
        for j in range(4):
            tp = psumz.tile([P, 1], DT.float32, tag="tcol", space="PSUM")
            nc.tensor.matmul(tp[:], z1b[:, j * P:(j + 1) * P], one11_t[:],
                             start=True, stop=True)
            nc.vector.tensor_copy(z1c[:, j:j + 1], tp[:])
        z2p = psumz.tile([1, FC2], DT.float32, tag="zacc", space="PSUM")
        for j in range(4):
            nc.tensor.matmul(z2p[:], z1c[:, j:j + 1],
                             we2_t[:, j * P:(j + 1) * P],
                             start=(j == 0), stop=(j == 3))
        z2s = sm.tile([1, FC2], DT.float32, tag="z2s")
        nc.vector.tensor_tensor(z2s[:], z2p[:], be2_t[:], op=ALU.add)
        z2b = sm.tile([1, FC2], DT.float32, tag="z2b")
        nc.scalar.activation(z2b[:], z2s[:], ACT.Relu)
        tp = psumz.tile([P, 1], DT.float32, tag="tcol", space="PSUM")
        nc.tensor.matmul(tp[:], z2b[:], one11_t[:], start=True, stop=True)
        z2c = sm.tile([P, 1], DT.float32, tag="z2c")
        nc.vector.tensor_copy(z2c[:], tp[:])
        z3p = psumz.tile([1, 1], DT.float32, tag="zacc", space="PSUM")
        nc.tensor.matmul(z3p[:], z2c[:], we3_t[:], start=True, stop=True)
        z3s = sm.tile([1, 1], DT.float32, tag="z3s")
        nc.vector.tensor_tensor(z3s[:], z3p[:], be3_t[:], op=ALU.add)
        nc.sync.dma_start(pred_d.ap(), z3s[:])


def _build_program(SPC):
    nc = bacc.Bacc("TRN2", target_bir_lowering=False, debug=False,
                   enable_asserts=False, num_devices=B)
    ins = {}
    for nm, shape, dt in _input_specs(SPC):
        ins[nm] = nc.dram_tensor(nm, shape, dt, kind="ExternalInput").ap()
    pred_ap = nc.dram_tensor("pred", [1, 1], DT.float32,
                             kind="ExternalOutput").ap()
    with tile.TileContext(nc) as tc:
        _trace_program(tc, ins, pred_ap, SPC)
    nc.compile()
    return nc


_PROG_CACHE = {}


def _get_program(SPC):
    if SPC not in _PROG_CACHE:
        _PROG_CACHE[SPC] = _build_program(SPC)
    return _PROG_CACHE[SPC]


def _prep_host(x, edge_index):
    """Split into per-graph shards, build slot-space index structures."""
    x = np.asarray(x, dtype=np.float32)
    ei = np.asarray(edge_index)
    src_all = ei[0].astype(np.int64)
    dst_all = ei[1].astype(np.int64)

    graphs = []
    per_chunk_counts = np.zeros((B, NCH), np.int64)
    for g in range(B):
        base = g * NUM_NODES
        m = slice(g * NUM_NODES * 32, (g + 1) * NUM_NODES * 32)
        src = src_all[m] - base
        dst = dst_all[m] - base
        loops = np.arange(NUM_NODES, dtype=np.int64)
        es = _slot(np.concatenate([src, loops]))
        ed = _slot(np.concatenate([dst, loops]))
        order = np.argsort(ed, kind="stable")
        es, ed = es[order], ed[order]
        ch = ed // P
        for c in range(NCH):
            per_chunk_counts[g, c] = int((ch == c).sum())
        graphs.append((es, ed, ch))

    SPC = int(np.ceil(per_chunk_counts.max() / P))
    NSLOTS = NCH * SPC

    shards = []
    for g in range(B):
        es, ed, ch = graphs[g]
        es_p = np.full(NSLOTS * P, 127, np.int64)
        ed_p = np.zeros(NSLOTS * P, np.int64)
        for c in range(NCH):
            sel = ch == c
            cnt = int(sel.sum())
            beg = c * SPC * P
            es_p[beg:beg + cnt] = es[sel]
            ed_p[beg:beg + cnt] = ed[sel]
            ed_p[beg + cnt:(c + 1) * SPC * P] = c * P + 127
        ew_src = np.tile(es_p.reshape(-1, 16).T.astype(np.int16), (8, 1))
        ew_dst = np.tile(ed_p.reshape(-1, 16).T.astype(np.int16), (8, 1))
        dl = (ed_p % P).astype(np.float32)
        dstloc = np.ascontiguousarray(dl.reshape(NSLOTS, P).T)

        xg = x[g * NUM_NODES:(g + 1) * NUM_NODES]
        xpad = np.zeros((NPAD, IN), np.float32)
        xpad[_slot(np.arange(NUM_NODES))] = xg
        shards.append(dict(xpad=xpad, ewsrc=ew_src, ewdst=ew_dst,
                           dstloc=dstloc))
    return SPC, shards


def _const_inputs(inp):
    """Replicated weight/const arrays keyed by dram tensor name."""
    def bcast_row(vec, dtype=np.float32):
        return np.ascontiguousarray(
            np.broadcast_to(np.asarray(vec, np.float32).reshape(-1), (P, P))
        ).astype(dtype)

    d = {}
    d["iota"] = np.ascontiguousarray(
        np.broadcast_to(np.arange(P, dtype=np.float32), (P, P))).astype(BF16)
    d["idf"] = np.eye(P, dtype=np.float32)
    d["idb"] = np.eye(P, dtype=np.float32).astype(BF16)
    d["one11"] = np.ones((1, 1), np.float32)
    for nm, key in [("wl1T", "Wl1"), ("wr1T", "Wr1"),
                    ("wl2T", "Wl2"), ("wr2T", "Wr2")]:
        d[nm] = np.ascontiguousarray(
            np.asarray(inp[key], np.float32).T)
    for nm, key in [("bl1b", "bl1"), ("br1b", "br1"), ("bias1b", "bias1"),
                    ("bl2b", "bl2"), ("br2b", "br2"), ("bias2b", "bias2")]:
        d[nm] = bcast_row(inp[key])
    d["att1b"] = bcast_row(np.asarray(inp["att1"], np.float32).reshape(-1), BF16)
    d["att2b"] = bcast_row(np.asarray(inp["att2"], np.float32).reshape(-1), BF16)
    d["wp1b"] = bcast_row(np.asarray(inp["Wp1"], np.float32).reshape(-1))
    d["wp2b"] = bcast_row(np.asarray(inp["Wp2"], np.float32).reshape(-1))

    def col_slot(vec, fill=0.0):
        v = np.full(NPAD, fill, np.float32)
        v[_slot(np.arange(NUM_NODES))] = np.asarray(vec, np.float32)
        return np.ascontiguousarray(v.reshape(NCH, P).T)

    d["lng"] = col_slot(inp["ln_g"])
    d["lnbt"] = col_slot(inp["ln_b"])
    d["maskc"] = col_slot(np.ones(NUM_NODES, np.float32))

    # We1 [512, 6000] -> slot space [3*2048, 512] -> [128, 48*512]
    we1 = np.asarray(inp["We1"], np.float32)
    we1s = np.zeros((3 * NPAD, FC1), np.float32)
    sl = _slot(np.arange(NUM_NODES))
    for ell in range(3):
        we1s[ell * NPAD + sl] = we1[:, ell * NUM_NODES:(ell + 1) * NUM_NODES].T
    we1ts = np.zeros((P, 48 * FC1), np.float32)
    for k in range(48):
        we1ts[:, k * FC1:(k + 1) * FC1] = we1s[k * P:(k + 1) * P]
    d["we1ts"] = we1ts.astype(BF16)
    d["be1r"] = np.asarray(inp["be1"], np.float32).reshape(1, FC1)
    d["we2t"] = np.ascontiguousarray(
        np.asarray(inp["We2"], np.float32).T.reshape(4, P, FC2)
        .transpose(1, 0, 2).reshape(P, 4 * FC2))
    d["be2r"] = np.asarray(inp["be2"], np.float32).reshape(1, FC2)
    d["we3t"] = np.asarray(inp["We3"], np.float32).T.reshape(P, 1)
    d["be3r"] = np.asarray(inp["be3"], np.float32).reshape(1, 1)
    return d


def kernel(**inputs) -> np.ndarray:
    SPC, shards = _prep_host(inputs["x"], inputs["edge_index"])
    consts = _const_inputs(inputs)
    nc = _get_program(SPC)
    in_maps = []
    for g in range(B):
        m = dict(consts)
        m.update(shards[g])
        in_maps.append(m)
    res = bass_utils.run_bass_kernel_spmd(nc, in_maps, core_ids=list(range(B)))
    pred = np.concatenate([r["pred"].reshape(1, 1) for r in res.results], 0)
    return pred.astype(np.float32)





# revision 9
# speedup vs baseline: 1.1771x; 1.1771x over previous
"""GATv2 (2-layer) + pooling + LayerNorm + MLP encoder — Trainium2 Bass kernel.

Sharding: data-parallel over graphs — 8 graphs, one per NeuronCore. All
conv/pool/FC weights replicated. Inside each core everything runs per-graph
(2000 nodes, 64000 edges + 2000 self-loops).

Node slot space: 16 chunks x 128 slots; real node n -> slot 128*(n//125) +
(n%125), so every chunk has 3 spare slots used as scatter targets for edge
padding. Edges are sorted by destination chunk and padded per chunk to a
uniform SPC slots (128 edges each) so one SPMD program serves all graphs.

Per edge-slot (128 edges across partitions, features along free):
  gather xl[src], xr[dst] rows via gpsimd.dma_gather (DRAM bf16 tables),
  v = xl_s + xr_d; u = leaky_relu(v); t = u * att; logits = reduce_C(t);
  exp -> alpha-numerators; W = xl_s * exp; aggregation + softmax denominator
  via a single PE matmul per slot: onehot(dst)^T @ [W | exp] accumulated in
  PSUM per chunk; epilogue divides by denominator, adds bias, relu.
"""

import sys

sys.path.insert(0, "/opt/trn_rl_repo")

from contextlib import ExitStack

import ml_dtypes
import numpy as np

import concourse.bass as bass
import concourse.bass_isa as bass_isa
import concourse.tile as tile
from concourse import bacc, bass_utils, mybir

BF16 = ml_dtypes.bfloat16
F32 = np.float32
DT = mybir.dt
ALU = mybir.AluOpType
ACT = mybir.ActivationFunctionType

NUM_NODES = 2000
B = 8
IN = 128
NH, C = 4, 32
HC = 128
NEG = 0.2
EPS = 1e-5
FC1, FC2 = 512, 128
P = 128
NCH = 16          # node chunks
CPN = 125         # real nodes per chunk
NPAD = NCH * P    # 2048 slots


def _slot(n):
    return (n // CPN) * P + (n % CPN)


def _ap(a, dims):
    return bass.AP(a.tensor, a.offset, dims)


def _bcast_mid(a, count):
    """[P, F] -> [P, count, F] with step-0 middle dim."""
    return bass.AP(a.tensor, a.offset,
                   [list(a.ap[0]), [0, count], list(a.ap[1])])


def _head_view(a, inner_pitch):
    """[..., 128]-inner AP -> [..., 4, 32] head view."""
    dims = [list(d) for d in a.ap[:-1]] + [[C, NH], [1, C]]
    assert a.ap[-1][1] == HC and a.ap[-1][0] == 1, list(a.ap[-1])
    return bass.AP(a.tensor, a.offset, dims)


def _exp_bcast(a):
    """[..., 4]-inner AP -> [..., 4, 32] with step-0 C dim."""
    dims = [list(d) for d in a.ap[:-1]] + [[1, NH], [0, C]]
    assert a.ap[-1][1] == NH
    return bass.AP(a.tensor, a.offset, dims)


INPUT_SPECS = None  # set below


def _input_specs(SPC):
    NSLOTS = NCH * SPC
    return [
        ("xpad", [NPAD, IN], DT.float32),
        ("ewsrc", [P, NSLOTS * 8], DT.int16),
        ("ewdst", [P, NSLOTS * 8], DT.int16),
        ("dstloc", [P, NSLOTS], DT.bfloat16),
        ("iota", [P, P], DT.bfloat16),
        ("idf", [P, P], DT.float32),
        ("idb", [P, P], DT.bfloat16),
        ("one11", [1, 1], DT.float32),
        ("wl1T", [P, P], DT.float32), ("wr1T", [P, P], DT.float32),
        ("wl2T", [P, P], DT.float32), ("wr2T", [P, P], DT.float32),
        ("bl1b", [P, P], DT.float32), ("br1b", [P, P], DT.float32),
        ("bias1b", [P, P], DT.float32), ("bl2b", [P, P], DT.float32),
        ("br2b", [P, P], DT.float32), ("bias2b", [P, P], DT.float32),
        ("att1b", [P, P], DT.bfloat16), ("att2b", [P, P], DT.bfloat16),
        ("wp1b", [P, P], DT.float32), ("wp2b", [P, P], DT.float32),
        ("lng", [P, NCH], DT.float32),
        ("lnbt", [P, NCH], DT.float32),
        ("maskc", [P, NCH], DT.float32),
        ("we1ts", [P, 48 * FC1], DT.bfloat16),
        ("be1r", [1, FC1], DT.float32),
        ("we2t", [P, 4 * P], DT.float32),
        ("be2r", [1, FC2], DT.float32),
        ("we3t", [P, 1], DT.float32),
        ("be3r", [1, 1], DT.float32),
    ]


def _trace_program(tc, ins, pred_ap, SPC, stop_after=None):
    """ins: dict name -> dram AP; pred_ap: output dram AP."""
    nc = tc.nc
    NSLOTS = NCH * SPC

    class _H:  # minimal handle-like wrapper so .ap() works uniformly
        def __init__(self, ap):
            self._ap = ap

        def ap(self):
            return self._ap

    xpad_d = _H(ins["xpad"])
    ewsrc_d = _H(ins["ewsrc"])
    ewdst_d = _H(ins["ewdst"])
    dstloc_d = _H(ins["dstloc"])
    iota_d = _H(ins["iota"])
    idf_d = _H(ins["idf"])
    idb_d = _H(ins["idb"])
    one11_d = _H(ins["one11"])
    wts = {nm: _H(ins[nm])
           for nm in ["wl1T", "wr1T", "wl2T", "wr2T", "bl1b", "br1b",
                      "bias1b", "bl2b", "br2b", "bias2b", "att1b", "att2b",
                      "wp1b", "wp2b"]}
    lng_d = _H(ins["lng"])
    lnb_d = _H(ins["lnbt"])
    mask_d = _H(ins["maskc"])
    we1_d = _H(ins["we1ts"])
    be1_d = _H(ins["be1r"])
    we2_d = _H(ins["we2t"])
    be2_d = _H(ins["be2r"])
    we3_d = _H(ins["we3t"])
    be3_d = _H(ins["be3r"])
    pred_d = _H(pred_ap)

    # internal DRAM gather tables
    tbl = {nm: nc.dram_tensor(f"{nm}_scr", [NPAD, HC], DT.bfloat16,
                              kind="Internal")
           for nm in ["tl1", "tr1", "tl2", "tr2"]}

    with ExitStack() as ctx:
        pers = ctx.enter_context(tc.tile_pool(name="pers", bufs=1))
        mega = ctx.enter_context(tc.tile_pool(name="mega", bufs=2))
        sm = ctx.enter_context(tc.tile_pool(name="sm", bufs=2))
        psum = ctx.enter_context(tc.tile_pool(name="psum", bufs=2, space="PSUM"))
        psumz = ctx.enter_context(tc.tile_pool(name="psumz", bufs=1, space="PSUM"))

        def load(d, shape, dt, tag):
            t = pers.tile(shape, dt, tag=tag)
            nc.sync.dma_start(t[:], d.ap())
            return t

        # ---- constant loads
        iota_t = load(iota_d, [P, P], DT.bfloat16, "iota")
        idf_t = load(idf_d, [P, P], DT.float32, "idf")
        one11_t = load(one11_d, [1, 1], DT.float32, "one11")
        w = {}
        for nm, d in wts.items():
            dt_w = DT.bfloat16 if nm in ("att1b", "att2b") else DT.float32
            w[nm] = load(d, [P, P], dt_w, nm)
        lng_t = load(lng_d, [P, NCH], DT.float32, "lng")
        lnb_t = load(lnb_d, [P, NCH], DT.float32, "lnbt")
        mask_t = load(mask_d, [P, NCH], DT.float32, "maskc")
        we1_t = load(we1_d, [P, 48 * FC1], DT.bfloat16, "we1")
        be1_t = load(be1_d, [1, FC1], DT.float32, "be1")
        we2_t = load(we2_d, [P, 4 * P], DT.float32, "we2")
        be2_t = load(be2_d, [1, FC2], DT.float32, "be2")
        we3_t = load(we3_d, [P, 1], DT.float32, "we3")
        be3_t = load(be3_d, [1, 1], DT.float32, "be3")
        ewsrc_t = load(ewsrc_d, [P, NSLOTS * 8], DT.int16, "ewsrc")
        ewdst_t = load(ewdst_d, [P, NSLOTS * 8], DT.int16, "ewdst")
        dstloc_t = load(dstloc_d, [P, NSLOTS], DT.bfloat16, "dstloc")

        # ---- x load [p, c, f]
        x_t = pers.tile([P, NCH, IN], DT.float32, tag="x")
        nc.sync.dma_start(x_t[:], xpad_d.ap().rearrange("(c p) f -> p c f", p=P))

        # ---- x0 pooling column + xT
        x0col = pers.tile([P, NCH], DT.float32, tag="x0col")
        x1col = pers.tile([P, NCH], DT.float32, tag="x1col")
        x2col = pers.tile([P, NCH], DT.float32, tag="x2col")
        xT_t = pers.tile([P, NCH, IN], DT.float32, tag="xT")
        for c in range(NCH):
            nc.vector.tensor_reduce(x0col[:, c:c + 1], x_t[:, c, :],
                                    axis=mybir.AxisListType.X, op=ALU.add)
            tp = psum.tile([P, P], DT.float32, tag="tp", space="PSUM")
            nc.tensor.transpose(tp[:], x_t[:, c, :], idf_t[:])
            nc.vector.tensor_copy(xT_t[:, c, :], tp[:])
        nc.vector.tensor_scalar(x0col[:], x0col[:], 1.0 / IN, None, op0=ALU.mult)

        def make_tables(lhsT_tile, wl, bl, wr, br, dl, dr):
            for c in range(NCH):
                for wmat, bvec, dst in ((wl, bl, dl), (wr, br, dr)):
                    pm = psum.tile([P, P], DT.float32, tag="tp", space="PSUM")
                    nc.tensor.matmul(pm[:], lhsT_tile[:, c, :], wmat[:],
                                     start=True, stop=True)
                    tb = sm.tile([P, P], DT.bfloat16, tag="tbl")
                    nc.vector.tensor_tensor(tb[:], pm[:], bvec[:], op=ALU.add)
                    nc.sync.dma_start(dst.ap()[c * P:(c + 1) * P, :], tb[:])

        def _finish():
            zf = sm.tile([1, 1], DT.float32, tag="zfin")
            nc.vector.memset(zf[:], 0.0)
            nc.sync.dma_start(pred_d.ap(), zf[:])

        if stop_after == "A0":
            _finish(); return

        make_tables(xT_t, w["wl1T"], w["bl1b"], w["wr1T"], w["br1b"],
                    tbl["tl1"], tbl["tr1"])
        if stop_after == "A":
            _finish(); return

        MSL = 32  # slots per gather mega (4096 idx = SWDGE ring limit)

        def gat_layer(tblL, tblR, att_t, biasO_t, h_out, nch=NCH,
                      skip_gather=False, gather_only=False):
            nslots = nch * SPC
            nmega = (nslots + MSL - 1) // MSL
            wexp_tiles = {}
            agg_tiles = {}
            for m in range(nmega):
                s0 = m * MSL
                ns = min(MSL, nslots - s0)
                nidx = ns * P
                srcg = mega.tile([P, ns, HC], DT.bfloat16, tag="srcg")
                dstg = mega.tile([P, ns, HC], DT.bfloat16, tag="dstg")
                if skip_gather:
                    nc.vector.memset(srcg[:], 0.25)
                    nc.vector.memset(dstg[:], 0.25)
                else:
                    nc.gpsimd.dma_gather(
                        srcg[:], tblL.ap(), ewsrc_t[:, s0 * 8:(s0 + ns) * 8],
                        nidx, nidx, elem_size=HC, queue_num=0,
                        single_packet=False)
                    nc.gpsimd.dma_gather(
                        dstg[:], tblR.ap(), ewdst_t[:, s0 * 8:(s0 + ns) * 8],
                        nidx, nidx, elem_size=HC, queue_num=0,
                        single_packet=False)
                if gather_only:
                    continue
                # one-hot megatile: oh[p, sl, d] = (iota[p, d] == dstloc[p, s0+sl])
                oh_mega = mega.tile([P, ns, P], DT.bfloat16, tag="ohm")
                iota_b = bass.AP(iota_t[:].tensor, iota_t[:].offset,
                                 [list(iota_t[:].ap[0]), [0, ns], [1, P]])
                dl = dstloc_t[:, s0:s0 + ns]
                dl_b = bass.AP(dl.tensor, dl.offset,
                               [list(dl.ap[0]), list(dl.ap[1]), [0, P]])
                nc.vector.tensor_tensor(oh_mega[:], iota_b, dl_b,
                                        op=ALU.is_equal)
                # edge features computed in-place inside wexp[:, :, 0:HC]
                wexp = mega.tile([P, ns, HC + NH], DT.bfloat16, tag="wexp")
                v = wexp[:, :, 0:HC]
                nc.vector.tensor_tensor(v, srcg[:], dstg[:], op=ALU.add)
                nc.scalar.activation(v, v, ACT.Prelu, alpha=NEG)
                nc.vector.tensor_tensor(v, v, _bcast_mid(att_t[:], ns),
                                        op=ALU.mult)
                lg = mega.tile([P, ns, NH], DT.float32, tag="lg")
                nc.vector.tensor_reduce(lg[:], _head_view(v, HC + NH),
                                        axis=mybir.AxisListType.X, op=ALU.add)
                nc.scalar.activation(wexp[:, :, HC:HC + NH], lg[:], ACT.Exp)
                nc.vector.tensor_tensor(
                    _head_view(wexp[:, :, 0:HC], HC + NH),
                    _head_view(srcg[:], HC),
                    _exp_bcast(wexp[:, :, HC:HC + NH]),
                    op=ALU.mult)
                wexp_tiles[m] = wexp
                # aggregation for the slots in this mega
                for sl in range(ns):
                    s = s0 + sl
                    c = s // SPC
                    if s == c * SPC:
                        agg = psum.tile([P, HC + NH], DT.float32,
                                        tag="agg", space="PSUM")
                        agg_tiles[c] = agg
                    agg = agg_tiles[c]
                    nc.tensor.matmul(agg[:], oh_mega[:, sl, :], wexp[:, sl, :],
                                     start=(s == c * SPC),
                                     stop=(s == (c + 1) * SPC - 1))
                    if s == (c + 1) * SPC - 1:
                        # epilogue: h = relu(agg / den + bias)
                        den = sm.tile([P, NH], DT.float32, tag="den")
                        nc.vector.tensor_scalar(den[:], agg[:, HC:HC + NH],
                                                1e-6, None, op0=ALU.add)
                        rec = sm.tile([P, NH], DT.float32, tag="rec")
                        nc.vector.reciprocal_approx_fast(rec[:], den[:])
                        t1 = sm.tile([P, P], DT.float32, tag="t1")
                        nc.vector.tensor_tensor(
                            _head_view(t1[:], P),
                            _head_view(agg[:, 0:HC], HC + NH),
                            _exp_bcast(rec[:]), op=ALU.mult)
                        t2 = sm.tile([P, P], DT.float32, tag="t2")
                        nc.vector.tensor_tensor(t2[:], t1[:], biasO_t[:],
                                                op=ALU.add)
                        nc.scalar.activation(h_out[:, c, :], t2[:], ACT.Relu)
                        del agg_tiles[c]

        if stop_after in ("G1", "G16", "NG1", "NG16", "C1"):
            hx = pers.tile([P, NCH, HC], DT.float32, tag="h1")
            n = 1 if stop_after in ("G1", "NG1", "C1") else NCH
            gat_layer(tbl["tl1"], tbl["tr1"], w["att1b"], w["bias1b"], hx,
                      nch=n, skip_gather=stop_after.startswith("NG"),
                      gather_only=stop_after.startswith("G"))
            _finish(); return

        h1_t = pers.tile([P, NCH, HC], DT.float32, tag="h1")
        gat_layer(tbl["tl1"], tbl["tr1"], w["att1b"], w["bias1b"], h1_t)
        if stop_after == "L1":
            _finish(); return

        # pooling x1, h1 transpose, layer-2 tables
        def pool_into(h_tile, wp_t, outcol):
            for c in range(NCH):
                pt = sm.tile([P, P], DT.float32, tag="ptmp")
                nc.vector.tensor_tensor(pt[:], h_tile[:, c, :], wp_t[:],
                                        op=ALU.mult)
                nc.vector.tensor_reduce(outcol[:, c:c + 1], pt[:],
                                        axis=mybir.AxisListType.X, op=ALU.add)

        pool_into(h1_t, w["wp1b"], x1col)
        h1T_t = pers.tile([P, NCH, HC], DT.float32, tag="h1T")
        for c in range(NCH):
            tp = psum.tile([P, P], DT.float32, tag="tp", space="PSUM")
            nc.tensor.transpose(tp[:], h1_t[:, c, :], idf_t[:])
            nc.vector.tensor_copy(h1T_t[:, c, :], tp[:])
        make_tables(h1T_t, w["wl2T"], w["bl2b"], w["wr2T"], w["br2b"],
                    tbl["tl2"], tbl["tr2"])
        if stop_after == "MID":
            _finish(); return

        h2_t = pers.tile([P, NCH, HC], DT.float32, tag="h2")
        gat_layer(tbl["tl2"], tbl["tr2"], w["att2b"], w["bias2b"], h2_t)
        pool_into(h2_t, w["wp2b"], x2col)
        if stop_after == "L2":
            _finish(); return

        # ---- LayerNorm on the three pooled rows -> bf16 columns [P, 48]
        lncols = pers.tile([P, 3 * NCH], DT.float32, tag="lncols")

        def layer_norm(xcol, colbase):
            xm = sm.tile([P, NCH], DT.float32, tag="xm")
            nc.vector.tensor_tensor(xm[:], xcol[:], mask_t[:], op=ALU.mult)
            alr = sm.tile([P, NCH], DT.float32, tag="alr")
            nc.gpsimd.partition_all_reduce(alr[:], xm[:], P,
                                           bass_isa.ReduceOp.add)
            tot = sm.tile([P, 1], DT.float32, tag="tot")
            nc.vector.tensor_reduce(tot[:], alr[:],
                                    axis=mybir.AxisListType.X, op=ALU.add)
            mean = sm.tile([P, 1], DT.float32, tag="mean")
            nc.vector.tensor_scalar(mean[:], tot[:], 1.0 / NUM_NODES, None,
                                    op0=ALU.mult)
            sq = sm.tile([P, NCH], DT.float32, tag="sq")
            nc.vector.tensor_tensor(sq[:], xm[:], xm[:], op=ALU.mult)
            alr2 = sm.tile([P, NCH], DT.float32, tag="alr2")
            nc.gpsimd.partition_all_reduce(alr2[:], sq[:], P,
                                           bass_isa.ReduceOp.add)
            tot2 = sm.tile([P, 1], DT.float32, tag="tot2")
            nc.vector.tensor_reduce(tot2[:], alr2[:],
                                    axis=mybir.AxisListType.X, op=ALU.add)
            msq = sm.tile([P, 1], DT.float32, tag="msq")
            nc.vector.tensor_scalar(msq[:], tot2[:], 1.0 / NUM_NODES, None,
                                    op0=ALU.mult)
            m2 = sm.tile([P, 1], DT.float32, tag="m2")
            nc.vector.tensor_tensor(m2[:], mean[:], mean[:], op=ALU.mult)
            var = sm.tile([P, 1], DT.float32, tag="var")
            nc.vector.tensor_tensor(var[:], msq[:], m2[:], op=ALU.subtract)
            sd = sm.tile([P, 1], DT.float32, tag="sd")
            nc.scalar.activation(sd[:], var[:], ACT.Sqrt, bias=eps_t[:])
            rstd = sm.tile([P, 1], DT.float32, tag="rstd")
            nc.vector.reciprocal_approx_fast(rstd[:], sd[:])
            rg = sm.tile([P, NCH], DT.float32, tag="rg")
            nc.vector.tensor_scalar(rg[:], lng_t[:], rstd[:], None,
                                    op0=ALU.mult)
            lnv = sm.tile([P, NCH], DT.float32, tag="lnv")
            nc.vector.scalar_tensor_tensor(lnv[:], xm[:], mean[:], rg[:],
                                           op0=ALU.subtract, op1=ALU.mult)
            nc.vector.tensor_tensor(lncols[:, colbase:colbase + NCH],
                                    lnv[:], lnb_t[:], op=ALU.add)

        eps_t = pers.tile([P, 1], DT.float32, tag="eps")
        nc.vector.memset(eps_t[:], EPS)

        layer_norm(x0col, 0)
        layer_norm(x1col, NCH)
        layer_norm(x2col, 2 * NCH)
        if stop_after == "LN":
            _finish(); return

        # ---- FC encoder
        lnbf = pers.tile([P, 3 * NCH], DT.bfloat16, tag="lnbf")
        nc.vector.tensor_copy(lnbf[:], lncols[:])
        z1p = psumz.tile([1, FC1], DT.float32, tag="zacc", space="PSUM")
        for k in range(48):
            nc.tensor.matmul(z1p[:], lnbf[:, k:k + 1],
                             we1_t[:, k * FC1:(k + 1) * FC1],
                             start=(k == 0), stop=(k == 47))
        z1s = sm.tile([1, FC1], DT.float32, tag="z1s")
        nc.vector.tensor_tensor(z1s[:], z1p[:], be1_t[:], op=ALU.add)
        z1b = sm.tile([1, FC1], DT.float32, tag="z1b")
        nc.scalar.activation(z1b[:], z1s[:], ACT.Relu)
        z1c = sm.tile([P, 4], DT.float32, tag="z1c")
        for j in range(4):
            tp = psumz.tile([P, 1], DT.float32, tag="tcol", space="PSUM")
            nc.tensor.matmul(tp[:], z1b[:, j * P:(j + 1) * P], one11_t[:],
                             start=True, stop=True)
            nc.vector.tensor_copy(z1c[:, j:j + 1], tp[:])
        z2p = psumz.tile([1, FC2], DT.float32, tag="zacc", space="PSUM")
        for j in range(4):
            nc.tensor.matmul(z2p[:], z1c[:, j:j + 1],
                             we2_t[:, j * P:(j + 1) * P],
                             start=(j == 0), stop=(j == 3))
        z2s = sm.tile([1, FC2], DT.float32, tag="z2s")
        nc.vector.tensor_tensor(z2s[:], z2p[:], be2_t[:], op=ALU.add)
        z2b = sm.tile([1, FC2], DT.float32, tag="z2b")
        nc.scalar.activation(z2b[:], z2s[:], ACT.Relu)
        tp = psumz.tile([P, 1], DT.float32, tag="tcol", space="PSUM")
        nc.tensor.matmul(tp[:], z2b[:], one11_t[:], start=True, stop=True)
        z2c = sm.tile([P, 1], DT.float32, tag="z2c")
        nc.vector.tensor_copy(z2c[:], tp[:])
        z3p = psumz.tile([1, 1], DT.float32, tag="zacc", space="PSUM")
        nc.tensor.matmul(z3p[:], z2c[:], we3_t[:], start=True, stop=True)
        z3s = sm.tile([1, 1], DT.float32, tag="z3s")
        nc.vector.tensor_tensor(z3s[:], z3p[:], be3_t[:], op=ALU.add)
        nc.sync.dma_start(pred_d.ap(), z3s[:])


def _build_program(SPC):
    nc = bacc.Bacc("TRN2", target_bir_lowering=False, debug=False,
                   enable_asserts=False, num_devices=B)
    ins = {}
    for nm, shape, dt in _input_specs(SPC):
        ins[nm] = nc.dram_tensor(nm, shape, dt, kind="ExternalInput").ap()
    pred_ap = nc.dram_tensor("pred", [1, 1], DT.float32,
                             kind="ExternalOutput").ap()
    with tile.TileContext(nc) as tc:
        _trace_program(tc, ins, pred_ap, SPC)
    nc.compile()
    return nc


_PROG_CACHE = {}


def _get_program(SPC):
    if SPC not in _PROG_CACHE:
        _PROG_CACHE[SPC] = _build_program(SPC)
    return _PROG_CACHE[SPC]


def _prep_host(x, edge_index):
    """Split into per-graph shards, build slot-space index structures."""
    x = np.asarray(x, dtype=np.float32)
    ei = np.asarray(edge_index)
    src_all = ei[0].astype(np.int64)
    dst_all = ei[1].astype(np.int64)

    graphs = []
    per_chunk_counts = np.zeros((B, NCH), np.int64)
    for g in range(B):
        base = g * NUM_NODES
        m = slice(g * NUM_NODES * 32, (g + 1) * NUM_NODES * 32)
        src = src_all[m] - base
        dst = dst_all[m] - base
        loops = np.arange(NUM_NODES, dtype=np.int64)
        es = _slot(np.concatenate([src, loops]))
        ed = _slot(np.concatenate([dst, loops]))
        order = np.argsort(ed, kind="stable")
        es, ed = es[order], ed[order]
        ch = ed // P
        for c in range(NCH):
            per_chunk_counts[g, c] = int((ch == c).sum())
        graphs.append((es, ed, ch))

    SPC = int(np.ceil(per_chunk_counts.max() / P))
    NSLOTS = NCH * SPC

    shards = []
    for g in range(B):
        es, ed, ch = graphs[g]
        es_p = np.full(NSLOTS * P, 127, np.int64)
        ed_p = np.zeros(NSLOTS * P, np.int64)
        for c in range(NCH):
            sel = ch == c
            cnt = int(sel.sum())
            beg = c * SPC * P
            es_p[beg:beg + cnt] = es[sel]
            ed_p[beg:beg + cnt] = ed[sel]
            ed_p[beg + cnt:(c + 1) * SPC * P] = c * P + 127
        ew_src = np.tile(es_p.reshape(-1, 16).T.astype(np.int16), (8, 1))
        ew_dst = np.tile(ed_p.reshape(-1, 16).T.astype(np.int16), (8, 1))
        dl = (ed_p % P).astype(np.float32)
        dstloc = np.ascontiguousarray(dl.reshape(NSLOTS, P).T).astype(BF16)

        xg = x[g * NUM_NODES:(g + 1) * NUM_NODES]
        xpad = np.zeros((NPAD, IN), np.float32)
        xpad[_slot(np.arange(NUM_NODES))] = xg
        shards.append(dict(xpad=xpad, ewsrc=ew_src, ewdst=ew_dst,
                           dstloc=dstloc))
    return SPC, shards


def _const_inputs(inp):
    """Replicated weight/const arrays keyed by dram tensor name."""
    def bcast_row(vec, dtype=np.float32):
        return np.ascontiguousarray(
            np.broadcast_to(np.asarray(vec, np.float32).reshape(-1), (P, P))
        ).astype(dtype)

    d = {}
    d["iota"] = np.ascontiguousarray(
        np.broadcast_to(np.arange(P, dtype=np.float32), (P, P))).astype(BF16)
    d["idf"] = np.eye(P, dtype=np.float32)
    d["idb"] = np.eye(P, dtype=np.float32).astype(BF16)
    d["one11"] = np.ones((1, 1), np.float32)
    for nm, key in [("wl1T", "Wl1"), ("wr1T", "Wr1"),
                    ("wl2T", "Wl2"), ("wr2T", "Wr2")]:
        d[nm] = np.ascontiguousarray(
            np.asarray(inp[key], np.float32).T)
    for nm, key in [("bl1b", "bl1"), ("br1b", "br1"), ("bias1b", "bias1"),
                    ("bl2b", "bl2"), ("br2b", "br2"), ("bias2b", "bias2")]:
        d[nm] = bcast_row(inp[key])
    d["att1b"] = bcast_row(np.asarray(inp["att1"], np.float32).reshape(-1), BF16)
    d["att2b"] = bcast_row(np.asarray(inp["att2"], np.float32).reshape(-1), BF16)
    d["wp1b"] = bcast_row(np.asarray(inp["Wp1"], np.float32).reshape(-1))
    d["wp2b"] = bcast_row(np.asarray(inp["Wp2"], np.float32).reshape(-1))

    def col_slot(vec, fill=0.0):
        v = np.full(NPAD, fill, np.float32)
        v[_slot(np.arange(NUM_NODES))] = np.asarray(vec, np.float32)
        return np.ascontiguousarray(v.reshape(NCH, P).T)

    d["lng"] = col_slot(inp["ln_g"])
    d["lnbt"] = col_slot(inp["ln_b"])
    d["maskc"] = col_slot(np.ones(NUM_NODES, np.float32))

    # We1 [512, 6000] -> slot space [3*2048, 512] -> [128, 48*512]
    we1 = np.asarray(inp["We1"], np.float32)
    we1s = np.zeros((3 * NPAD, FC1), np.float32)
    sl = _slot(np.arange(NUM_NODES))
    for ell in range(3):
        we1s[ell * NPAD + sl] = we1[:, ell * NUM_NODES:(ell + 1) * NUM_NODES].T
    we1ts = np.zeros((P, 48 * FC1), np.float32)
    for k in range(48):
        we1ts[:, k * FC1:(k + 1) * FC1] = we1s[k * P:(k + 1) * P]
    d["we1ts"] = we1ts.astype(BF16)
    d["be1r"] = np.asarray(inp["be1"], np.float32).reshape(1, FC1)
    d["we2t"] = np.ascontiguousarray(
        np.asarray(inp["We2"], np.float32).T.reshape(4, P, FC2)
        .transpose(1, 0, 2).reshape(P, 4 * FC2))
    d["be2r"] = np.asarray(inp["be2"], np.float32).reshape(1, FC2)
    d["we3t"] = np.asarray(inp["We3"], np.float32).T.reshape(P, 1)
    d["be3r"] = np.asarray(inp["be3"], np.float32).reshape(1, 1)
    return d


def kernel(**inputs) -> np.ndarray:
    SPC, shards = _prep_host(inputs["x"], inputs["edge_index"])
    consts = _const_inputs(inputs)
    nc = _get_program(SPC)
    in_maps = []
    for g in range(B):
        m = dict(consts)
        m.update(shards[g])
        in_maps.append(m)
    res = bass_utils.run_bass_kernel_spmd(nc, in_maps, core_ids=list(range(B)))
    pred = np.concatenate([r["pred"].reshape(1, 1) for r in res.results], 0)
    return pred.astype(np.float32)





# revision 11
# speedup vs baseline: 2.0176x; 1.7140x over previous
"""GATv2 (2-layer) + pooling + LayerNorm + MLP encoder — Trainium2 Bass kernel.

Sharding: data-parallel over graphs — 8 graphs, one per NeuronCore. All
conv/pool/FC weights replicated. Inside each core everything runs per-graph
(2000 nodes, 64000 edges + 2000 self-loops).

Node slot space: 16 chunks x 128 slots; real node n -> slot 128*(n//125) +
(n%125), so every chunk has 3 spare slots used as scatter targets for edge
padding. Edges are sorted by destination chunk and padded per chunk to a
uniform SPC slots (128 edges each) so one SPMD program serves all graphs.

Per edge-slot (128 edges across partitions, features along free):
  gather xl[src], xr[dst] rows via gpsimd.dma_gather (DRAM bf16 tables),
  v = xl_s + xr_d; u = leaky_relu(v); t = u * att; logits = reduce_C(t);
  exp -> alpha-numerators; W = xl_s * exp; aggregation + softmax denominator
  via a single PE matmul per slot: onehot(dst)^T @ [W | exp] accumulated in
  PSUM per chunk; epilogue divides by denominator, adds bias, relu.
"""

import sys

sys.path.insert(0, "/opt/trn_rl_repo")

from contextlib import ExitStack

import ml_dtypes
import numpy as np

import concourse.bass as bass
import concourse.bass_isa as bass_isa
import concourse.tile as tile
from concourse import bacc, bass_utils, mybir

BF16 = ml_dtypes.bfloat16
F32 = np.float32
DT = mybir.dt
ALU = mybir.AluOpType
ACT = mybir.ActivationFunctionType

NUM_NODES = 2000
B = 8
IN = 128
NH, C = 4, 32
HC = 128
NEG = 0.2
EPS = 1e-5
FC1, FC2 = 512, 128
P = 128
NCH = 16          # node chunks
CPN = 125         # real nodes per chunk
NPAD = NCH * P    # 2048 slots


def _slot(n):
    return (n // CPN) * P + (n % CPN)


def _ap(a, dims):
    return bass.AP(a.tensor, a.offset, dims)


def _bcast_mid(a, count):
    """[P, F] -> [P, count, F] with step-0 middle dim."""
    return bass.AP(a.tensor, a.offset,
                   [list(a.ap[0]), [0, count], list(a.ap[1])])


def _head_view(a, inner_pitch):
    """[..., 128]-inner AP -> [..., 4, 32] head view."""
    dims = [list(d) for d in a.ap[:-1]] + [[C, NH], [1, C]]
    assert a.ap[-1][1] == HC and a.ap[-1][0] == 1, list(a.ap[-1])
    return bass.AP(a.tensor, a.offset, dims)


def _exp_bcast(a):
    """[..., 4]-inner AP -> [..., 4, 32] with step-0 C dim."""
    dims = [list(d) for d in a.ap[:-1]] + [[1, NH], [0, C]]
    assert a.ap[-1][1] == NH
    return bass.AP(a.tensor, a.offset, dims)


INPUT_SPECS = None  # set below


def _input_specs(SPC):
    NSLOTS = NCH * SPC
    return [
        ("xpad", [NPAD, IN], DT.float32),
        ("ewsrc", [P, NSLOTS * 8], DT.int16),
        ("ewdst", [P, NSLOTS * 8], DT.int16),
        ("dstloc", [P, NSLOTS], DT.bfloat16),
        ("iota", [P, P], DT.bfloat16),
        ("idf", [P, P], DT.float32),
        ("idb", [P, P], DT.bfloat16),
        ("one11", [1, 1], DT.float32),
        ("wl1T", [P, P], DT.float32), ("wr1T", [P, P], DT.float32),
        ("wl2T", [P, P], DT.float32), ("wr2T", [P, P], DT.float32),
        ("bl1b", [P, P], DT.float32), ("br1b", [P, P], DT.float32),
        ("bias1b", [P, P], DT.float32), ("bl2b", [P, P], DT.float32),
        ("br2b", [P, P], DT.float32), ("bias2b", [P, P], DT.float32),
        ("att1b", [P, P], DT.bfloat16), ("att2b", [P, P], DT.bfloat16),
        ("wp1b", [P, P], DT.float32), ("wp2b", [P, P], DT.float32),
        ("lng", [P, NCH], DT.float32),
        ("lnbt", [P, NCH], DT.float32),
        ("maskc", [P, NCH], DT.float32),
        ("we1ts", [P, 48 * FC1], DT.bfloat16),
        ("be1r", [1, FC1], DT.float32),
        ("we2t", [P, 4 * P], DT.float32),
        ("be2r", [1, FC2], DT.float32),
        ("we3t", [P, 1], DT.float32),
        ("be3r", [1, 1], DT.float32),
    ]


def _trace_program(tc, ins, pred_ap, SPC, stop_after=None):
    """ins: dict name -> dram AP; pred_ap: output dram AP."""
    nc = tc.nc
    NSLOTS = NCH * SPC

    class _H:  # minimal handle-like wrapper so .ap() works uniformly
        def __init__(self, ap):
            self._ap = ap

        def ap(self):
            return self._ap

    xpad_d = _H(ins["xpad"])
    ewsrc_d = _H(ins["ewsrc"])
    ewdst_d = _H(ins["ewdst"])
    dstloc_d = _H(ins["dstloc"])
    iota_d = _H(ins["iota"])
    idf_d = _H(ins["idf"])
    idb_d = _H(ins["idb"])
    one11_d = _H(ins["one11"])
    wts = {nm: _H(ins[nm])
           for nm in ["wl1T", "wr1T", "wl2T", "wr2T", "bl1b", "br1b",
                      "bias1b", "bl2b", "br2b", "bias2b", "att1b", "att2b",
                      "wp1b", "wp2b"]}
    lng_d = _H(ins["lng"])
    lnb_d = _H(ins["lnbt"])
    mask_d = _H(ins["maskc"])
    we1_d = _H(ins["we1ts"])
    be1_d = _H(ins["be1r"])
    we2_d = _H(ins["we2t"])
    be2_d = _H(ins["be2r"])
    we3_d = _H(ins["we3t"])
    be3_d = _H(ins["be3r"])
    pred_d = _H(pred_ap)

    # internal DRAM gather tables
    tbl = {nm: nc.dram_tensor(f"{nm}_scr", [NPAD, HC], DT.bfloat16,
                              kind="Internal")
           for nm in ["tl1", "tr1", "tl2", "tr2"]}

    with ExitStack() as ctx:
        pers = ctx.enter_context(tc.tile_pool(name="pers", bufs=1))
        mega = ctx.enter_context(tc.tile_pool(name="mega", bufs=2))
        sm = ctx.enter_context(tc.tile_pool(name="sm", bufs=2))
        psum = ctx.enter_context(tc.tile_pool(name="psum", bufs=2, space="PSUM"))
        psumz = ctx.enter_context(tc.tile_pool(name="psumz", bufs=1, space="PSUM"))

        def load(d, shape, dt, tag):
            t = pers.tile(shape, dt, tag=tag)
            nc.sync.dma_start(t[:], d.ap())
            return t

        # ---- constant loads
        iota_t = load(iota_d, [P, P], DT.bfloat16, "iota")
        idf_t = load(idf_d, [P, P], DT.float32, "idf")
        one11_t = load(one11_d, [1, 1], DT.float32, "one11")
        w = {}
        for nm, d in wts.items():
            dt_w = DT.bfloat16 if nm in ("att1b", "att2b") else DT.float32
            w[nm] = load(d, [P, P], dt_w, nm)
        lng_t = load(lng_d, [P, NCH], DT.float32, "lng")
        lnb_t = load(lnb_d, [P, NCH], DT.float32, "lnbt")
        mask_t = load(mask_d, [P, NCH], DT.float32, "maskc")
        we1_t = load(we1_d, [P, 48 * FC1], DT.bfloat16, "we1")
        be1_t = load(be1_d, [1, FC1], DT.float32, "be1")
        we2_t = load(we2_d, [P, 4 * P], DT.float32, "we2")
        be2_t = load(be2_d, [1, FC2], DT.float32, "be2")
        we3_t = load(we3_d, [P, 1], DT.float32, "we3")
        be3_t = load(be3_d, [1, 1], DT.float32, "be3")
        ewsrc_t = load(ewsrc_d, [P, NSLOTS * 8], DT.int16, "ewsrc")
        ewdst_t = load(ewdst_d, [P, NSLOTS * 8], DT.int16, "ewdst")
        dstloc_t = load(dstloc_d, [P, NSLOTS], DT.bfloat16, "dstloc")

        # ---- x load [p, c, f]
        x_t = pers.tile([P, NCH, IN], DT.float32, tag="x")
        nc.sync.dma_start(x_t[:], xpad_d.ap().rearrange("(c p) f -> p c f", p=P))

        # ---- x0 pooling column + xT
        x0col = pers.tile([P, NCH], DT.float32, tag="x0col")
        x1col = pers.tile([P, NCH], DT.float32, tag="x1col")
        x2col = pers.tile([P, NCH], DT.float32, tag="x2col")
        xT_t = pers.tile([P, NCH, IN], DT.float32, tag="xT")
        for c in range(NCH):
            nc.vector.tensor_reduce(x0col[:, c:c + 1], x_t[:, c, :],
                                    axis=mybir.AxisListType.X, op=ALU.add)
            tp = psum.tile([P, P], DT.float32, tag="tp", space="PSUM")
            nc.tensor.transpose(tp[:], x_t[:, c, :], idf_t[:])
            nc.vector.tensor_copy(xT_t[:, c, :], tp[:])
        nc.vector.tensor_scalar(x0col[:], x0col[:], 1.0 / IN, None, op0=ALU.mult)

        def make_tables(lhsT_tile, wl, bl, wr, br, dl, dr):
            for c in range(NCH):
                for wmat, bvec, dst in ((wl, bl, dl), (wr, br, dr)):
                    pm = psum.tile([P, P], DT.float32, tag="tp", space="PSUM")
                    nc.tensor.matmul(pm[:], lhsT_tile[:, c, :], wmat[:],
                                     start=True, stop=True)
                    tb = sm.tile([P, P], DT.bfloat16, tag="tbl")
                    nc.vector.tensor_tensor(tb[:], pm[:], bvec[:], op=ALU.add)
                    nc.sync.dma_start(dst.ap()[c * P:(c + 1) * P, :], tb[:])

        def _finish():
            zf = sm.tile([1, 1], DT.float32, tag="zfin")
            nc.vector.memset(zf[:], 0.0)
            nc.sync.dma_start(pred_d.ap(), zf[:])

        if stop_after == "A0":
            _finish(); return

        make_tables(xT_t, w["wl1T"], w["bl1b"], w["wr1T"], w["br1b"],
                    tbl["tl1"], tbl["tr1"])
        if stop_after == "A":
            _finish(); return

        MSL = 32  # slots per gather mega (4096 idx = SWDGE ring limit)

        def gat_layer(tblL, tblR, att_t, biasO_t, h_out, nch=NCH,
                      skip_gather=False, gather_only=False):
            nslots = nch * SPC
            nmega = (nslots + MSL - 1) // MSL
            wexp_tiles = {}
            agg_tiles = {}
            for m in range(nmega):
                s0 = m * MSL
                ns = min(MSL, nslots - s0)
                nidx = ns * P
                srcg = mega.tile([P, ns, HC], DT.bfloat16, tag="srcg")
                dstg = mega.tile([P, ns, HC], DT.bfloat16, tag="dstg")
                if skip_gather:
                    nc.vector.memset(srcg[:], 0.25)
                    nc.vector.memset(dstg[:], 0.25)
                else:
                    nc.gpsimd.dma_gather(
                        srcg[:], tblL.ap(), ewsrc_t[:, s0 * 8:(s0 + ns) * 8],
                        nidx, nidx, elem_size=HC, queue_num=(2 * m) % 4,
                        single_packet=False)
                    nc.gpsimd.dma_gather(
                        dstg[:], tblR.ap(), ewdst_t[:, s0 * 8:(s0 + ns) * 8],
                        nidx, nidx, elem_size=HC, queue_num=(2 * m + 1) % 4,
                        single_packet=False)
                if gather_only:
                    continue
                # one-hot megatile: oh[p, sl, d] = (iota[p, d] == dstloc[p, s0+sl])
                oh_mega = mega.tile([P, ns, P], DT.bfloat16, tag="ohm")
                iota_b = bass.AP(iota_t[:].tensor, iota_t[:].offset,
                                 [list(iota_t[:].ap[0]), [0, ns], [1, P]])
                dl = dstloc_t[:, s0:s0 + ns]
                dl_b = bass.AP(dl.tensor, dl.offset,
                               [list(dl.ap[0]), list(dl.ap[1]), [0, P]])
                nc.vector.tensor_tensor(oh_mega[:], iota_b, dl_b,
                                        op=ALU.is_equal)
                # edge features computed in-place inside wexp[:, :, 0:HC]
                wexp = mega.tile([P, ns, HC + NH], DT.bfloat16, tag="wexp")
                v = wexp[:, :, 0:HC]
                nc.vector.tensor_tensor(v, srcg[:], dstg[:], op=ALU.add)
                nc.scalar.activation(v, v, ACT.Prelu, alpha=NEG)
                nc.vector.tensor_tensor(v, v, _bcast_mid(att_t[:], ns),
                                        op=ALU.mult)
                lg = mega.tile([P, ns, NH], DT.float32, tag="lg")
                nc.vector.tensor_reduce(lg[:], _head_view(v, HC + NH),
                                        axis=mybir.AxisListType.X, op=ALU.add)
                nc.scalar.activation(wexp[:, :, HC:HC + NH], lg[:], ACT.Exp)
                nc.vector.tensor_tensor(
                    _head_view(wexp[:, :, 0:HC], HC + NH),
                    _head_view(srcg[:], HC),
                    _exp_bcast(wexp[:, :, HC:HC + NH]),
                    op=ALU.mult)
                wexp_tiles[m] = wexp
                # aggregation for the slots in this mega
                for sl in range(ns):
                    s = s0 + sl
                    c = s // SPC
                    if s == c * SPC:
                        agg = psum.tile([P, HC + NH], DT.float32,
                                        tag="agg", space="PSUM")
                        agg_tiles[c] = agg
                    agg = agg_tiles[c]
                    nc.tensor.matmul(agg[:], oh_mega[:, sl, :], wexp[:, sl, :],
                                     start=(s == c * SPC),
                                     stop=(s == (c + 1) * SPC - 1))
                    if s == (c + 1) * SPC - 1:
                        # epilogue: h = relu(agg / den + bias)
                        den = sm.tile([P, NH], DT.float32, tag="den")
                        nc.vector.tensor_scalar(den[:], agg[:, HC:HC + NH],
                                                1e-6, None, op0=ALU.add)
                        rec = sm.tile([P, NH], DT.float32, tag="rec")
                        nc.vector.reciprocal_approx_fast(rec[:], den[:])
                        t1 = sm.tile([P, P], DT.float32, tag="t1")
                        nc.vector.tensor_tensor(
                            _head_view(t1[:], P),
                            _head_view(agg[:, 0:HC], HC + NH),
                            _exp_bcast(rec[:]), op=ALU.mult)
                        t2 = sm.tile([P, P], DT.float32, tag="t2")
                        nc.vector.tensor_tensor(t2[:], t1[:], biasO_t[:],
                                                op=ALU.add)
                        nc.scalar.activation(h_out[:, c, :], t2[:], ACT.Relu)
                        del agg_tiles[c]

        if stop_after in ("G1", "G16", "NG1", "NG16", "C1"):
            hx = pers.tile([P, NCH, HC], DT.float32, tag="h1")
            n = 1 if stop_after in ("G1", "NG1", "C1") else NCH
            gat_layer(tbl["tl1"], tbl["tr1"], w["att1b"], w["bias1b"], hx,
                      nch=n, skip_gather=stop_after.startswith("NG"),
                      gather_only=stop_after.startswith("G"))
            _finish(); return

        h1_t = pers.tile([P, NCH, HC], DT.float32, tag="h1")
        gat_layer(tbl["tl1"], tbl["tr1"], w["att1b"], w["bias1b"], h1_t)
        if stop_after == "L1":
            _finish(); return

        # pooling x1, h1 transpose, layer-2 tables
        def pool_into(h_tile, wp_t, outcol):
            for c in range(NCH):
                pt = sm.tile([P, P], DT.float32, tag="ptmp")
                nc.vector.tensor_tensor(pt[:], h_tile[:, c, :], wp_t[:],
                                        op=ALU.mult)
                nc.vector.tensor_reduce(outcol[:, c:c + 1], pt[:],
                                        axis=mybir.AxisListType.X, op=ALU.add)

        pool_into(h1_t, w["wp1b"], x1col)
        h1T_t = pers.tile([P, NCH, HC], DT.float32, tag="h1T")
        for c in range(NCH):
            tp = psum.tile([P, P], DT.float32, tag="tp", space="PSUM")
            nc.tensor.transpose(tp[:], h1_t[:, c, :], idf_t[:])
            nc.vector.tensor_copy(h1T_t[:, c, :], tp[:])
        make_tables(h1T_t, w["wl2T"], w["bl2b"], w["wr2T"], w["br2b"],
                    tbl["tl2"], tbl["tr2"])
        if stop_after == "MID":
            _finish(); return

        h2_t = pers.tile([P, NCH, HC], DT.float32, tag="h2")
        gat_layer(tbl["tl2"], tbl["tr2"], w["att2b"], w["bias2b"], h2_t)
        pool_into(h2_t, w["wp2b"], x2col)
        if stop_after == "L2":
            _finish(); return

        # ---- LayerNorm on the three pooled rows -> bf16 columns [P, 48]
        lncols = pers.tile([P, 3 * NCH], DT.float32, tag="lncols")

        def layer_norm(xcol, colbase):
            xm = sm.tile([P, NCH], DT.float32, tag="xm")
            nc.vector.tensor_tensor(xm[:], xcol[:], mask_t[:], op=ALU.mult)
            alr = sm.tile([P, NCH], DT.float32, tag="alr")
            nc.gpsimd.partition_all_reduce(alr[:], xm[:], P,
                                           bass_isa.ReduceOp.add)
            tot = sm.tile([P, 1], DT.float32, tag="tot")
            nc.vector.tensor_reduce(tot[:], alr[:],
                                    axis=mybir.AxisListType.X, op=ALU.add)
            mean = sm.tile([P, 1], DT.float32, tag="mean")
            nc.vector.tensor_scalar(mean[:], tot[:], 1.0 / NUM_NODES, None,
                                    op0=ALU.mult)
            sq = sm.tile([P, NCH], DT.float32, tag="sq")
            nc.vector.tensor_tensor(sq[:], xm[:], xm[:], op=ALU.mult)
            alr2 = sm.tile([P, NCH], DT.float32, tag="alr2")
            nc.gpsimd.partition_all_reduce(alr2[:], sq[:], P,
                                           bass_isa.ReduceOp.add)
            tot2 = sm.tile([P, 1], DT.float32, tag="tot2")
            nc.vector.tensor_reduce(tot2[:], alr2[:],
                                    axis=mybir.AxisListType.X, op=ALU.add)
            msq = sm.tile([P, 1], DT.float32, tag="msq")
            nc.vector.tensor_scalar(msq[:], tot2[:], 1.0 / NUM_NODES, None,
                                    op0=ALU.mult)
            m2 = sm.tile([P, 1], DT.float32, tag="m2")
            nc.vector.tensor_tensor(m2[:], mean[:], mean[:], op=ALU.mult)
            var = sm.tile([P, 1], DT.float32, tag="var")
            nc.vector.tensor_tensor(var[:], msq[:], m2[:], op=ALU.subtract)
            sd = sm.tile([P, 1], DT.float32, tag="sd")
            nc.scalar.activation(sd[:], var[:], ACT.Sqrt, bias=eps_t[:])
            rstd = sm.tile([P, 1], DT.float32, tag="rstd")
            nc.vector.reciprocal_approx_fast(rstd[:], sd[:])
            rg = sm.tile([P, NCH], DT.float32, tag="rg")
            nc.vector.tensor_scalar(rg[:], lng_t[:], rstd[:], None,
                                    op0=ALU.mult)
            lnv = sm.tile([P, NCH], DT.float32, tag="lnv")
            nc.vector.scalar_tensor_tensor(lnv[:], xm[:], mean[:], rg[:],
                                           op0=ALU.subtract, op1=ALU.mult)
            nc.vector.tensor_tensor(lncols[:, colbase:colbase + NCH],
                                    lnv[:], lnb_t[:], op=ALU.add)

        eps_t = pers.tile([P, 1], DT.float32, tag="eps")
        nc.vector.memset(eps_t[:], EPS)

        layer_norm(x0col, 0)
        layer_norm(x1col, NCH)
        layer_norm(x2col, 2 * NCH)
        if stop_after == "LN":
            _finish(); return

        # ---- FC encoder
        lnbf = pers.tile([P, 3 * NCH], DT.bfloat16, tag="lnbf")
        nc.vector.tensor_copy(lnbf[:], lncols[:])
        z1p = psumz.tile([1, FC1], DT.float32, tag="zacc", space="PSUM")
        for k in range(48):
            nc.tensor.matmul(z1p[:], lnbf[:, k:k + 1],
                             we1_t[:, k * FC1:(k + 1) * FC1],
                             start=(k == 0), stop=(k == 47))
        z1s = sm.tile([1, FC1], DT.float32, tag="z1s")
        nc.vector.tensor_tensor(z1s[:], z1p[:], be1_t[:], op=ALU.add)
        z1b = sm.tile([1, FC1], DT.float32, tag="z1b")
        nc.scalar.activation(z1b[:], z1s[:], ACT.Relu)
        z1c = sm.tile([P, 4], DT.float32, tag="z1c")
        for j in range(4):
            tp = psumz.tile([P, 1], DT.float32, tag="tcol", space="PSUM")
            nc.tensor.matmul(tp[:], z1b[:, j * P:(j + 1) * P], one11_t[:],
                             start=True, stop=True)
            nc.vector.tensor_copy(z1c[:, j:j + 1], tp[:])
        z2p = psumz.tile([1, FC2], DT.float32, tag="zacc", space="PSUM")
        for j in range(4):
            nc.tensor.matmul(z2p[:], z1c[:, j:j + 1],
                             we2_t[:, j * P:(j + 1) * P],
                             start=(j == 0), stop=(j == 3))
        z2s = sm.tile([1, FC2], DT.float32, tag="z2s")
        nc.vector.tensor_tensor(z2s[:], z2p[:], be2_t[:], op=ALU.add)
        z2b = sm.tile([1, FC2], DT.float32, tag="z2b")
        nc.scalar.activation(z2b[:], z2s[:], ACT.Relu)
        tp = psumz.tile([P, 1], DT.float32, tag="tcol", space="PSUM")
        nc.tensor.matmul(tp[:], z2b[:], one11_t[:], start=True, stop=True)
        z2c = sm.tile([P, 1], DT.float32, tag="z2c")
        nc.vector.tensor_copy(z2c[:], tp[:])
        z3p = psumz.tile([1, 1], DT.float32, tag="zacc", space="PSUM")
        nc.tensor.matmul(z3p[:], z2c[:], we3_t[:], start=True, stop=True)
        z3s = sm.tile([1, 1], DT.float32, tag="z3s")
        nc.vector.tensor_tensor(z3s[:], z3p[:], be3_t[:], op=ALU.add)
        nc.sync.dma_start(pred_d.ap(), z3s[:])


def _build_program(SPC):
    nc = bacc.Bacc("TRN2", target_bir_lowering=False, debug=False,
                   enable_asserts=False, num_devices=B, num_swdge_queues=4)
    ins = {}
    for nm, shape, dt in _input_specs(SPC):
        ins[nm] = nc.dram_tensor(nm, shape, dt, kind="ExternalInput").ap()
    pred_ap = nc.dram_tensor("pred", [1, 1], DT.float32,
                             kind="ExternalOutput").ap()
    with tile.TileContext(nc) as tc:
        _trace_program(tc, ins, pred_ap, SPC)
    nc.compile()
    return nc


_PROG_CACHE = {}


def _get_program(SPC):
    if SPC not in _PROG_CACHE:
        _PROG_CACHE[SPC] = _build_program(SPC)
    return _PROG_CACHE[SPC]


def _prep_host(x, edge_index):
    """Split into per-graph shards, build slot-space index structures."""
    x = np.asarray(x, dtype=np.float32)
    ei = np.asarray(edge_index)
    src_all = ei[0].astype(np.int64)
    dst_all = ei[1].astype(np.int64)

    graphs = []
    per_chunk_counts = np.zeros((B, NCH), np.int64)
    for g in range(B):
        base = g * NUM_NODES
        m = slice(g * NUM_NODES * 32, (g + 1) * NUM_NODES * 32)
        src = src_all[m] - base
        dst = dst_all[m] - base
        loops = np.arange(NUM_NODES, dtype=np.int64)
        es = _slot(np.concatenate([src, loops]))
        ed = _slot(np.concatenate([dst, loops]))
        order = np.argsort(ed, kind="stable")
        es, ed = es[order], ed[order]
        ch = ed // P
        for c in range(NCH):
            per_chunk_counts[g, c] = int((ch == c).sum())
        graphs.append((es, ed, ch))

    SPC = int(np.ceil(per_chunk_counts.max() / P))
    NSLOTS = NCH * SPC

    shards = []
    for g in range(B):
        es, ed, ch = graphs[g]
        es_p = np.full(NSLOTS * P, 127, np.int64)
        ed_p = np.zeros(NSLOTS * P, np.int64)
        for c in range(NCH):
            sel = ch == c
            cnt = int(sel.sum())
            beg = c * SPC * P
            es_p[beg:beg + cnt] = es[sel]
            ed_p[beg:beg + cnt] = ed[sel]
            ed_p[beg + cnt:(c + 1) * SPC * P] = c * P + 127
        ew_src = np.tile(es_p.reshape(-1, 16).T.astype(np.int16), (8, 1))
        ew_dst = np.tile(ed_p.reshape(-1, 16).T.astype(np.int16), (8, 1))
        dl = (ed_p % P).astype(np.float32)
        dstloc = np.ascontiguousarray(dl.reshape(NSLOTS, P).T).astype(BF16)

        xg = x[g * NUM_NODES:(g + 1) * NUM_NODES]
        xpad = np.zeros((NPAD, IN), np.float32)
        xpad[_slot(np.arange(NUM_NODES))] = xg
        shards.append(dict(xpad=xpad, ewsrc=ew_src, ewdst=ew_dst,
                           dstloc=dstloc))
    return SPC, shards


def _const_inputs(inp):
    """Replicated weight/const arrays keyed by dram tensor name."""
    def bcast_row(vec, dtype=np.float32):
        return np.ascontiguousarray(
            np.broadcast_to(np.asarray(vec, np.float32).reshape(-1), (P, P))
        ).astype(dtype)

    d = {}
    d["iota"] = np.ascontiguousarray(
        np.broadcast_to(np.arange(P, dtype=np.float32), (P, P))).astype(BF16)
    d["idf"] = np.eye(P, dtype=np.float32)
    d["idb"] = np.eye(P, dtype=np.float32).astype(BF16)
    d["one11"] = np.ones((1, 1), np.float32)
    for nm, key in [("wl1T", "Wl1"), ("wr1T", "Wr1"),
                    ("wl2T", "Wl2"), ("wr2T", "Wr2")]:
        d[nm] = np.ascontiguousarray(
            np.asarray(inp[key], np.float32).T)
    for nm, key in [("bl1b", "bl1"), ("br1b", "br1"), ("bias1b", "bias1"),
                    ("bl2b", "bl2"), ("br2b", "br2"), ("bias2b", "bias2")]:
        d[nm] = bcast_row(inp[key])
    d["att1b"] = bcast_row(np.asarray(inp["att1"], np.float32).reshape(-1), BF16)
    d["att2b"] = bcast_row(np.asarray(inp["att2"], np.float32).reshape(-1), BF16)
    d["wp1b"] = bcast_row(np.asarray(inp["Wp1"], np.float32).reshape(-1))
    d["wp2b"] = bcast_row(np.asarray(inp["Wp2"], np.float32).reshape(-1))

    def col_slot(vec, fill=0.0):
        v = np.full(NPAD, fill, np.float32)
        v[_slot(np.arange(NUM_NODES))] = np.asarray(vec, np.float32)
        return np.ascontiguousarray(v.reshape(NCH, P).T)

    d["lng"] = col_slot(inp["ln_g"])
    d["lnbt"] = col_slot(inp["ln_b"])
    d["maskc"] = col_slot(np.ones(NUM_NODES, np.float32))

    # We1 [512, 6000] -> slot space [3*2048, 512] -> [128, 48*512]
    we1 = np.asarray(inp["We1"], np.float32)
    we1s = np.zeros((3 * NPAD, FC1), np.float32)
    sl = _slot(np.arange(NUM_NODES))
    for ell in range(3):
        we1s[ell * NPAD + sl] = we1[:, ell * NUM_NODES:(ell + 1) * NUM_NODES].T
    we1ts = np.zeros((P, 48 * FC1), np.float32)
    for k in range(48):
        we1ts[:, k * FC1:(k + 1) * FC1] = we1s[k * P:(k + 1) * P]
    d["we1ts"] = we1ts.astype(BF16)
    d["be1r"] = np.asarray(inp["be1"], np.float32).reshape(1, FC1)
    d["we2t"] = np.ascontiguousarray(
        np.asarray(inp["We2"], np.float32).T.reshape(4, P, FC2)
        .transpose(1, 0, 2).reshape(P, 4 * FC2))
    d["be2r"] = np.asarray(inp["be2"], np.float32).reshape(1, FC2)
    d["we3t"] = np.asarray(inp["We3"], np.float32).T.reshape(P, 1)
    d["be3r"] = np.asarray(inp["be3"], np.float32).reshape(1, 1)
    return d


def kernel(**inputs) -> np.ndarray:
    SPC, shards = _prep_host(inputs["x"], inputs["edge_index"])
    consts = _const_inputs(inputs)
    nc = _get_program(SPC)
    in_maps = []
    for g in range(B):
        m = dict(consts)
        m.update(shards[g])
        in_maps.append(m)
    res = bass_utils.run_bass_kernel_spmd(nc, in_maps, core_ids=list(range(B)))
    pred = np.concatenate([r["pred"].reshape(1, 1) for r in res.results], 0)
    return pred.astype(np.float32)





# revision 17
# speedup vs baseline: 2.5890x; 1.2832x over previous
"""GATv2 (2-layer) + pooling + LayerNorm + MLP encoder — Trainium2 Bass kernel.

Sharding: data-parallel over graphs — 8 graphs, one per NeuronCore. All
conv/pool/FC weights replicated. Inside each core everything runs per-graph
(2000 nodes, 64000 edges + 2000 self-loops).

Node slot space: 16 chunks x 128 slots; real node n -> slot 128*(n//125) +
(n%125), so every chunk has 3 spare slots used as scatter targets for edge
padding. Edges are sorted by destination chunk and padded per chunk to a
uniform SPC slots (128 edges each) so one SPMD program serves all graphs.

Per edge-slot (128 edges across partitions, features along free):
  gather xl[src], xr[dst] rows via gpsimd.dma_gather (DRAM bf16 tables),
  v = xl_s + xr_d; u = leaky_relu(v); t = u * att; logits = reduce_C(t);
  exp -> alpha-numerators; W = xl_s * exp; aggregation + softmax denominator
  via a single PE matmul per slot: onehot(dst)^T @ [W | exp] accumulated in
  PSUM per chunk; epilogue divides by denominator, adds bias, relu.
"""

import sys

sys.path.insert(0, "/opt/trn_rl_repo")

from contextlib import ExitStack

import ml_dtypes
import numpy as np

import concourse.bass as bass
import concourse.bass_isa as bass_isa
import concourse.tile as tile
from concourse import bacc, bass_utils, mybir

BF16 = ml_dtypes.bfloat16
F32 = np.float32
DT = mybir.dt
ALU = mybir.AluOpType
ACT = mybir.ActivationFunctionType

NUM_NODES = 2000
B = 8
IN = 128
NH, C = 4, 32
HC = 128
NEG = 0.2
EPS = 1e-5
FC1, FC2 = 512, 128
P = 128
NCH = 16          # node chunks
CPN = 125         # real nodes per chunk
NPAD = NCH * P    # 2048 slots


def _slot(n):
    return (n // CPN) * P + (n % CPN)


def _ap(a, dims):
    return bass.AP(a.tensor, a.offset, dims)


def _bcast_mid(a, count):
    """[P, F] -> [P, count, F] with step-0 middle dim."""
    return bass.AP(a.tensor, a.offset,
                   [list(a.ap[0]), [0, count], list(a.ap[1])])


def _head_view(a, inner_pitch):
    """[..., 128]-inner AP -> [..., 4, 32] head view."""
    dims = [list(d) for d in a.ap[:-1]] + [[C, NH], [1, C]]
    assert a.ap[-1][1] == HC and a.ap[-1][0] == 1, list(a.ap[-1])
    return bass.AP(a.tensor, a.offset, dims)


def _exp_bcast(a):
    """[..., 4]-inner AP -> [..., 4, 32] with step-0 C dim."""
    dims = [list(d) for d in a.ap[:-1]] + [[1, NH], [0, C]]
    assert a.ap[-1][1] == NH
    return bass.AP(a.tensor, a.offset, dims)


INPUT_SPECS = None  # set below


def _input_specs(SPC):
    NSLOTS = NCH * SPC
    return [
        ("xpad", [NPAD, IN], DT.float32),
        ("ewsrc", [P, NSLOTS * 8], DT.int16),
        ("ewdst", [P, NSLOTS * 8], DT.int16),
        ("dstloc", [P, NSLOTS], DT.bfloat16),
        ("iota", [P, P], DT.bfloat16),
        ("idf", [P, P], DT.float32),
        ("idb", [P, P], DT.bfloat16),
        ("one11", [1, 1], DT.float32),
        ("wl1T", [P, P], DT.float32), ("wr1T", [P, P], DT.float32),
        ("wl2T", [P, P], DT.float32), ("wr2T", [P, P], DT.float32),
        ("bl1b", [P, P], DT.float32), ("br1b", [P, P], DT.float32),
        ("bias1b", [P, P], DT.float32), ("bl2b", [P, P], DT.float32),
        ("br2b", [P, P], DT.float32), ("bias2b", [P, P], DT.float32),
        ("att1b", [P, P], DT.bfloat16), ("att2b", [P, P], DT.bfloat16),
        ("wp1b", [P, P], DT.float32), ("wp2b", [P, P], DT.float32),
        ("lng", [P, NCH], DT.float32),
        ("lnbt", [P, NCH], DT.float32),
        ("maskc", [P, NCH], DT.float32),
        ("we1ts", [P, 48 * FC1], DT.bfloat16),
        ("be1r", [1, FC1], DT.float32),
        ("we2t", [P, 4 * P], DT.float32),
        ("be2r", [1, FC2], DT.float32),
        ("we3t", [P, 1], DT.float32),
        ("be3r", [1, 1], DT.float32),
    ]


def _trace_program(tc, ins, pred_ap, SPC, stop_after=None):
    """ins: dict name -> dram AP; pred_ap: output dram AP."""
    nc = tc.nc
    NSLOTS = NCH * SPC

    class _H:  # minimal handle-like wrapper so .ap() works uniformly
        def __init__(self, ap):
            self._ap = ap

        def ap(self):
            return self._ap

    xpad_d = _H(ins["xpad"])
    ewsrc_d = _H(ins["ewsrc"])
    ewdst_d = _H(ins["ewdst"])
    dstloc_d = _H(ins["dstloc"])
    iota_d = _H(ins["iota"])
    idf_d = _H(ins["idf"])
    idb_d = _H(ins["idb"])
    one11_d = _H(ins["one11"])
    wts = {nm: _H(ins[nm])
           for nm in ["wl1T", "wr1T", "wl2T", "wr2T", "bl1b", "br1b",
                      "bias1b", "bl2b", "br2b", "bias2b", "att1b", "att2b",
                      "wp1b", "wp2b"]}
    lng_d = _H(ins["lng"])
    lnb_d = _H(ins["lnbt"])
    mask_d = _H(ins["maskc"])
    we1_d = _H(ins["we1ts"])
    be1_d = _H(ins["be1r"])
    we2_d = _H(ins["we2t"])
    be2_d = _H(ins["be2r"])
    we3_d = _H(ins["we3t"])
    be3_d = _H(ins["be3r"])
    pred_d = _H(pred_ap)

    # internal DRAM gather tables
    tbl = {nm: nc.dram_tensor(f"{nm}_scr", [NPAD, HC], DT.bfloat16,
                              kind="Internal")
           for nm in ["tl1", "tr1", "tl2", "tr2"]}

    with ExitStack() as ctx:
        pers = ctx.enter_context(tc.tile_pool(name="pers", bufs=1))
        mega = ctx.enter_context(tc.tile_pool(name="mega", bufs=3))
        sm = ctx.enter_context(tc.tile_pool(name="sm", bufs=2))
        psum = ctx.enter_context(tc.tile_pool(name="psum", bufs=2, space="PSUM"))
        psumz = ctx.enter_context(tc.tile_pool(name="psumz", bufs=1, space="PSUM"))

        def load(d, shape, dt, tag):
            t = pers.tile(shape, dt, tag=tag)
            nc.sync.dma_start(t[:], d.ap())
            return t

        # ---- constant loads
        iota_t = load(iota_d, [P, P], DT.bfloat16, "iota")
        idf_t = load(idf_d, [P, P], DT.float32, "idf")
        one11_t = load(one11_d, [1, 1], DT.float32, "one11")
        w = {}
        for nm, d in wts.items():
            dt_w = DT.bfloat16 if nm in ("att1b", "att2b") else DT.float32
            w[nm] = load(d, [P, P], dt_w, nm)
        lng_t = load(lng_d, [P, NCH], DT.float32, "lng")
        lnb_t = load(lnb_d, [P, NCH], DT.float32, "lnbt")
        mask_t = load(mask_d, [P, NCH], DT.float32, "maskc")
        we1_t = load(we1_d, [P, 48 * FC1], DT.bfloat16, "we1")
        be1_t = load(be1_d, [1, FC1], DT.float32, "be1")
        we2_t = load(we2_d, [P, 4 * P], DT.float32, "we2")
        be2_t = load(be2_d, [1, FC2], DT.float32, "be2")
        we3_t = load(we3_d, [P, 1], DT.float32, "we3")
        be3_t = load(be3_d, [1, 1], DT.float32, "be3")
        dstloc_t = load(dstloc_d, [P, NSLOTS], DT.bfloat16, "dstloc")

        # ---- x load [p, c, f]
        x_t = pers.tile([P, NCH, IN], DT.float32, tag="x")
        nc.sync.dma_start(x_t[:], xpad_d.ap().rearrange("(c p) f -> p c f", p=P))

        # ---- x0 pooling column + xT
        x0col = pers.tile([P, NCH], DT.float32, tag="x0col")
        x1col = pers.tile([P, NCH], DT.float32, tag="x1col")
        x2col = pers.tile([P, NCH], DT.float32, tag="x2col")
        for c in range(NCH):
            nc.vector.tensor_reduce(x0col[:, c:c + 1], x_t[:, c, :],
                                    axis=mybir.AxisListType.X, op=ALU.add)
        nc.vector.tensor_scalar(x0col[:], x0col[:], 1.0 / IN, None, op0=ALU.mult)

        def make_tables(h_tile, wl, bl, wr, br, dl, dr):
            # per chunk: transpose h[:, c, :] on PE, then the two table matmuls
            for c in range(NCH):
                tp = psum.tile([P, P], DT.float32, tag="tp", space="PSUM")
                nc.tensor.transpose(tp[:], h_tile[:, c, :], idf_t[:])
                hT = sm.tile([P, P], DT.float32, tag="hTc")
                nc.vector.tensor_copy(hT[:], tp[:])
                for wmat, bvec, dst in ((wl, bl, dl), (wr, br, dr)):
                    pm = psum.tile([P, P], DT.float32, tag="tp", space="PSUM")
                    nc.tensor.matmul(pm[:], hT[:], wmat[:],
                                     start=True, stop=True)
                    tb = sm.tile([P, P], DT.bfloat16, tag="tbl")
                    nc.vector.tensor_tensor(tb[:], pm[:], bvec[:], op=ALU.add)
                    nc.sync.dma_start(dst.ap()[c * P:(c + 1) * P, :], tb[:])

        def _finish():
            zf = sm.tile([1, 1], DT.float32, tag="zfin")
            nc.vector.memset(zf[:], 0.0)
            nc.sync.dma_start(pred_d.ap(), zf[:])

        if stop_after == "A0":
            _finish(); return

        make_tables(x_t, w["wl1T"], w["bl1b"], w["wr1T"], w["br1b"],
                    tbl["tl1"], tbl["tr1"])
        if stop_after == "A":
            _finish(); return

        MSL = 32  # slots per gather mega (4096 idx = SWDGE ring limit)

        def gat_layer(tblL, tblR, att_t, biasO_t, h_out, nch=NCH,
                      skip_gather=False, gather_only=False):
            nslots = nch * SPC
            nmega = (nslots + MSL - 1) // MSL
            wexp_tiles = {}
            agg_tiles = {}
            for m in range(nmega):
                s0 = m * MSL
                ns = min(MSL, nslots - s0)
                nidx = ns * P
                srcg = mega.tile([P, ns, HC], DT.bfloat16, tag="srcg")
                dstg = mega.tile([P, ns, HC], DT.bfloat16, tag="dstg")
                if skip_gather:
                    nc.vector.memset(srcg[:], 0.25)
                    nc.vector.memset(dstg[:], 0.25)
                else:
                    esrc = mega.tile([P, ns * 8], DT.int16, tag="esrc")
                    nc.sync.dma_start(
                        esrc[:], ewsrc_d.ap()[:, s0 * 8:(s0 + ns) * 8])
                    edst = mega.tile([P, ns * 8], DT.int16, tag="edst")
                    nc.sync.dma_start(
                        edst[:], ewdst_d.ap()[:, s0 * 8:(s0 + ns) * 8])
                    nc.gpsimd.dma_gather(
                        srcg[:], tblL.ap(), esrc[:],
                        nidx, nidx, elem_size=HC, queue_num=(2 * m) % 4,
                        single_packet=False)
                    nc.gpsimd.dma_gather(
                        dstg[:], tblR.ap(), edst[:],
                        nidx, nidx, elem_size=HC, queue_num=(2 * m + 1) % 4,
                        single_packet=False)
                if gather_only:
                    continue
                # one-hot megatile: oh[p, sl, d] = (iota[p, d] == dstloc[p, s0+sl])
                oh_mega = mega.tile([P, ns, P], DT.bfloat16, tag="ohm")
                iota_b = bass.AP(iota_t[:].tensor, iota_t[:].offset,
                                 [list(iota_t[:].ap[0]), [0, ns], [1, P]])
                dl = dstloc_t[:, s0:s0 + ns]
                dl_b = bass.AP(dl.tensor, dl.offset,
                               [list(dl.ap[0]), list(dl.ap[1]), [0, P]])
                nc.vector.tensor_tensor(oh_mega[:], iota_b, dl_b,
                                        op=ALU.is_equal)
                # edge features computed in-place inside wexp[:, :, 0:HC]
                wexp = mega.tile([P, ns, HC + NH], DT.bfloat16, tag="wexp")
                v = wexp[:, :, 0:HC]
                nc.vector.tensor_tensor(v, srcg[:], dstg[:], op=ALU.add)
                nc.scalar.activation(v, v, ACT.Prelu, alpha=NEG)
                nc.vector.tensor_tensor(v, v, _bcast_mid(att_t[:], ns),
                                        op=ALU.mult)
                lg = mega.tile([P, ns, NH], DT.float32, tag="lg")
                nc.vector.tensor_reduce(lg[:], _head_view(v, HC + NH),
                                        axis=mybir.AxisListType.X, op=ALU.add)
                nc.scalar.activation(wexp[:, :, HC:HC + NH], lg[:], ACT.Exp)
                nc.vector.tensor_tensor(
                    _head_view(wexp[:, :, 0:HC], HC + NH),
                    _head_view(srcg[:], HC),
                    _exp_bcast(wexp[:, :, HC:HC + NH]),
                    op=ALU.mult)
                wexp_tiles[m] = wexp
                # aggregation for the slots in this mega
                for sl in range(ns):
                    s = s0 + sl
                    c = s // SPC
                    if s == c * SPC:
                        agg = psum.tile([P, HC + NH], DT.float32,
                                        tag="agg", space="PSUM")
                        agg_tiles[c] = agg
                    agg = agg_tiles[c]
                    nc.tensor.matmul(agg[:], oh_mega[:, sl, :], wexp[:, sl, :],
                                     start=(s == c * SPC),
                                     stop=(s == (c + 1) * SPC - 1))
                    if s == (c + 1) * SPC - 1:
                        # epilogue: h = relu(agg / den + bias)
                        den = sm.tile([P, NH], DT.float32, tag="den")
                        nc.vector.tensor_scalar(den[:], agg[:, HC:HC + NH],
                                                1e-6, None, op0=ALU.add)
                        rec = sm.tile([P, NH], DT.float32, tag="rec")
                        nc.vector.reciprocal_approx_fast(rec[:], den[:])
                        t1 = sm.tile([P, P], DT.float32, tag="t1")
                        nc.vector.tensor_tensor(
                            _head_view(t1[:], P),
                            _head_view(agg[:, 0:HC], HC + NH),
                            _exp_bcast(rec[:]), op=ALU.mult)
                        t2 = sm.tile([P, P], DT.float32, tag="t2")
                        nc.vector.tensor_tensor(t2[:], t1[:], biasO_t[:],
                                                op=ALU.add)
                        nc.scalar.activation(h_out[:, c, :], t2[:], ACT.Relu)
                        del agg_tiles[c]

        if stop_after in ("G1", "G16", "NG1", "NG16", "C1"):
            hx = pers.tile([P, NCH, HC], DT.float32, tag="h1")
            n = 1 if stop_after in ("G1", "NG1", "C1") else NCH
            gat_layer(tbl["tl1"], tbl["tr1"], w["att1b"], w["bias1b"], hx,
                      nch=n, skip_gather=stop_after.startswith("NG"),
                      gather_only=stop_after.startswith("G"))
            _finish(); return

        h1_t = pers.tile([P, NCH, HC], DT.float32, tag="h1")
        gat_layer(tbl["tl1"], tbl["tr1"], w["att1b"], w["bias1b"], h1_t)
        if stop_after == "L1":
            _finish(); return

        # pooling x1, h1 transpose, layer-2 tables
        def pool_into(h_tile, wp_t, outcol):
            for c in range(NCH):
                pt = sm.tile([P, P], DT.float32, tag="ptmp")
                nc.vector.tensor_tensor(pt[:], h_tile[:, c, :], wp_t[:],
                                        op=ALU.mult)
                nc.vector.tensor_reduce(outcol[:, c:c + 1], pt[:],
                                        axis=mybir.AxisListType.X, op=ALU.add)

        pool_into(h1_t, w["wp1b"], x1col)
        make_tables(h1_t, w["wl2T"], w["bl2b"], w["wr2T"], w["br2b"],
                    tbl["tl2"], tbl["tr2"])
        if stop_after == "MID":
            _finish(); return

        h2_t = pers.tile([P, NCH, HC], DT.float32, tag="h2")
        gat_layer(tbl["tl2"], tbl["tr2"], w["att2b"], w["bias2b"], h2_t)
        pool_into(h2_t, w["wp2b"], x2col)
        if stop_after == "L2":
            _finish(); return

        # ---- LayerNorm on the three pooled rows -> bf16 columns [P, 48]
        lncols = pers.tile([P, 3 * NCH], DT.float32, tag="lncols")

        def layer_norm(xcol, colbase):
            xm = sm.tile([P, NCH], DT.float32, tag="xm")
            nc.vector.tensor_tensor(xm[:], xcol[:], mask_t[:], op=ALU.mult)
            alr = sm.tile([P, NCH], DT.float32, tag="alr")
            nc.gpsimd.partition_all_reduce(alr[:], xm[:], P,
                                           bass_isa.ReduceOp.add)
            tot = sm.tile([P, 1], DT.float32, tag="tot")
            nc.vector.tensor_reduce(tot[:], alr[:],
                                    axis=mybir.AxisListType.X, op=ALU.add)
            mean = sm.tile([P, 1], DT.float32, tag="mean")
            nc.vector.tensor_scalar(mean[:], tot[:], 1.0 / NUM_NODES, None,
                                    op0=ALU.mult)
            sq = sm.tile([P, NCH], DT.float32, tag="sq")
            nc.vector.tensor_tensor(sq[:], xm[:], xm[:], op=ALU.mult)
            alr2 = sm.tile([P, NCH], DT.float32, tag="alr2")
            nc.gpsimd.partition_all_reduce(alr2[:], sq[:], P,
                                           bass_isa.ReduceOp.add)
            tot2 = sm.tile([P, 1], DT.float32, tag="tot2")
            nc.vector.tensor_reduce(tot2[:], alr2[:],
                                    axis=mybir.AxisListType.X, op=ALU.add)
            msq = sm.tile([P, 1], DT.float32, tag="msq")
            nc.vector.tensor_scalar(msq[:], tot2[:], 1.0 / NUM_NODES, None,
                                    op0=ALU.mult)
            m2 = sm.tile([P, 1], DT.float32, tag="m2")
            nc.vector.tensor_tensor(m2[:], mean[:], mean[:], op=ALU.mult)
            var = sm.tile([P, 1], DT.float32, tag="var")
            nc.vector.tensor_tensor(var[:], msq[:], m2[:], op=ALU.subtract)
            sd = sm.tile([P, 1], DT.float32, tag="sd")
            nc.scalar.activation(sd[:], var[:], ACT.Sqrt, bias=eps_t[:])
            rstd = sm.tile([P, 1], DT.float32, tag="rstd")
            nc.vector.reciprocal_approx_fast(rstd[:], sd[:])
            rg = sm.tile([P, NCH], DT.float32, tag="rg")
            nc.vector.tensor_scalar(rg[:], lng_t[:], rstd[:], None,
                                    op0=ALU.mult)
            lnv = sm.tile([P, NCH], DT.float32, tag="lnv")
            nc.vector.scalar_tensor_tensor(lnv[:], xm[:], mean[:], rg[:],
                                           op0=ALU.subtract, op1=ALU.mult)
            nc.vector.tensor_tensor(lncols[:, colbase:colbase + NCH],
                                    lnv[:], lnb_t[:], op=ALU.add)

        eps_t = pers.tile([P, 1], DT.float32, tag="eps")
        nc.vector.memset(eps_t[:], EPS)

        layer_norm(x0col, 0)
        layer_norm(x1col, NCH)
        layer_norm(x2col, 2 * NCH)
        if stop_after == "LN":
            _finish(); return

        # ---- FC encoder
        lnbf = pers.tile([P, 3 * NCH], DT.bfloat16, tag="lnbf")
        nc.vector.tensor_copy(lnbf[:], lncols[:])
        z1p = psumz.tile([1, FC1], DT.float32, tag="zacc", space="PSUM")
        for k in range(48):
            nc.tensor.matmul(z1p[:], lnbf[:, k:k + 1],
                             we1_t[:, k * FC1:(k + 1) * FC1],
                             start=(k == 0), stop=(k == 47))
        z1s = sm.tile([1, FC1], DT.float32, tag="z1s")
        nc.vector.tensor_tensor(z1s[:], z1p[:], be1_t[:], op=ALU.add)
        z1b = sm.tile([1, FC1], DT.float32, tag="z1b")
        nc.scalar.activation(z1b[:], z1s[:], ACT.Relu)
        z1c = sm.tile([P, 4], DT.float32, tag="z1c")
        for j in range(4):
            tp = psumz.tile([P, 1], DT.float32, tag="tcol", space="PSUM")
            nc.tensor.matmul(tp[:], z1b[:, j * P:(j + 1) * P], one11_t[:],
                             start=True, stop=True)
            nc.vector.tensor_copy(z1c[:, j:j + 1], tp[:])
        z2p = psumz.tile([1, FC2], DT.float32, tag="zacc", space="PSUM")
        for j in range(4):
            nc.tensor.matmul(z2p[:], z1c[:, j:j + 1],
                             we2_t[:, j * P:(j + 1) * P],
                             start=(j == 0), stop=(j == 3))
        z2s = sm.tile([1, FC2], DT.float32, tag="z2s")
        nc.vector.tensor_tensor(z2s[:], z2p[:], be2_t[:], op=ALU.add)
        z2b = sm.tile([1, FC2], DT.float32, tag="z2b")
        nc.scalar.activation(z2b[:], z2s[:], ACT.Relu)
        tp = psumz.tile([P, 1], DT.float32, tag="tcol", space="PSUM")
        nc.tensor.matmul(tp[:], z2b[:], one11_t[:], start=True, stop=True)
        z2c = sm.tile([P, 1], DT.float32, tag="z2c")
        nc.vector.tensor_copy(z2c[:], tp[:])
        z3p = psumz.tile([1, 1], DT.float32, tag="zacc", space="PSUM")
        nc.tensor.matmul(z3p[:], z2c[:], we3_t[:], start=True, stop=True)
        z3s = sm.tile([1, 1], DT.float32, tag="z3s")
        nc.vector.tensor_tensor(z3s[:], z3p[:], be3_t[:], op=ALU.add)
        nc.sync.dma_start(pred_d.ap(), z3s[:])


def _build_program(SPC):
    nc = bacc.Bacc("TRN2", target_bir_lowering=False, debug=False,
                   enable_asserts=False, num_devices=B, num_swdge_queues=4)
    ins = {}
    for nm, shape, dt in _input_specs(SPC):
        ins[nm] = nc.dram_tensor(nm, shape, dt, kind="ExternalInput").ap()
    pred_ap = nc.dram_tensor("pred", [1, 1], DT.float32,
                             kind="ExternalOutput").ap()
    with tile.TileContext(nc) as tc:
        _trace_program(tc, ins, pred_ap, SPC)
    nc.compile()
    return nc


_PROG_CACHE = {}


def _get_program(SPC):
    if SPC not in _PROG_CACHE:
        _PROG_CACHE[SPC] = _build_program(SPC)
    return _PROG_CACHE[SPC]


def _prep_host(x, edge_index):
    """Split into per-graph shards, build slot-space index structures."""
    x = np.asarray(x, dtype=np.float32)
    ei = np.asarray(edge_index)
    src_all = ei[0].astype(np.int64)
    dst_all = ei[1].astype(np.int64)

    graphs = []
    per_chunk_counts = np.zeros((B, NCH), np.int64)
    for g in range(B):
        base = g * NUM_NODES
        m = slice(g * NUM_NODES * 32, (g + 1) * NUM_NODES * 32)
        src = src_all[m] - base
        dst = dst_all[m] - base
        loops = np.arange(NUM_NODES, dtype=np.int64)
        es = _slot(np.concatenate([src, loops]))
        ed = _slot(np.concatenate([dst, loops]))
        order = np.argsort(ed, kind="stable")
        es, ed = es[order], ed[order]
        ch = ed // P
        for c in range(NCH):
            per_chunk_counts[g, c] = int((ch == c).sum())
        graphs.append((es, ed, ch))

    SPC = int(np.ceil(per_chunk_counts.max() / P))
    NSLOTS = NCH * SPC

    shards = []
    for g in range(B):
        es, ed, ch = graphs[g]
        es_p = np.full(NSLOTS * P, 127, np.int64)
        ed_p = np.zeros(NSLOTS * P, np.int64)
        for c in range(NCH):
            sel = ch == c
            cnt = int(sel.sum())
            beg = c * SPC * P
            es_p[beg:beg + cnt] = es[sel]
            ed_p[beg:beg + cnt] = ed[sel]
            ed_p[beg + cnt:(c + 1) * SPC * P] = c * P + 127
        ew_src = np.tile(es_p.reshape(-1, 16).T.astype(np.int16), (8, 1))
        ew_dst = np.tile(ed_p.reshape(-1, 16).T.astype(np.int16), (8, 1))
        dl = (ed_p % P).astype(np.float32)
        dstloc = np.ascontiguousarray(dl.reshape(NSLOTS, P).T).astype(BF16)

        xg = x[g * NUM_NODES:(g + 1) * NUM_NODES]
        xpad = np.zeros((NPAD, IN), np.float32)
        xpad[_slot(np.arange(NUM_NODES))] = xg
        shards.append(dict(xpad=xpad, ewsrc=ew_src, ewdst=ew_dst,
                           dstloc=dstloc))
    return SPC, shards


def _const_inputs(inp):
    """Replicated weight/const arrays keyed by dram tensor name."""
    def bcast_row(vec, dtype=np.float32):
        return np.ascontiguousarray(
            np.broadcast_to(np.asarray(vec, np.float32).reshape(-1), (P, P))
        ).astype(dtype)

    d = {}
    d["iota"] = np.ascontiguousarray(
        np.broadcast_to(np.arange(P, dtype=np.float32), (P, P))).astype(BF16)
    d["idf"] = np.eye(P, dtype=np.float32)
    d["idb"] = np.eye(P, dtype=np.float32).astype(BF16)
    d["one11"] = np.ones((1, 1), np.float32)
    for nm, key in [("wl1T", "Wl1"), ("wr1T", "Wr1"),
                    ("wl2T", "Wl2"), ("wr2T", "Wr2")]:
        d[nm] = np.ascontiguousarray(
            np.asarray(inp[key], np.float32).T)
    for nm, key in [("bl1b", "bl1"), ("br1b", "br1"), ("bias1b", "bias1"),
                    ("bl2b", "bl2"), ("br2b", "br2"), ("bias2b", "bias2")]:
        d[nm] = bcast_row(inp[key])
    d["att1b"] = bcast_row(np.asarray(inp["att1"], np.float32).reshape(-1), BF16)
    d["att2b"] = bcast_row(np.asarray(inp["att2"], np.float32).reshape(-1), BF16)
    d["wp1b"] = bcast_row(np.asarray(inp["Wp1"], np.float32).reshape(-1))
    d["wp2b"] = bcast_row(np.asarray(inp["Wp2"], np.float32).reshape(-1))

    def col_slot(vec, fill=0.0):
        v = np.full(NPAD, fill, np.float32)
        v[_slot(np.arange(NUM_NODES))] = np.asarray(vec, np.float32)
        return np.ascontiguousarray(v.reshape(NCH, P).T)

    d["lng"] = col_slot(inp["ln_g"])
    d["lnbt"] = col_slot(inp["ln_b"])
    d["maskc"] = col_slot(np.ones(NUM_NODES, np.float32))

    # We1 [512, 6000] -> slot space [3*2048, 512] -> [128, 48*512]
    we1 = np.asarray(inp["We1"], np.float32)
    we1s = np.zeros((3 * NPAD, FC1), np.float32)
    sl = _slot(np.arange(NUM_NODES))
    for ell in range(3):
        we1s[ell * NPAD + sl] = we1[:, ell * NUM_NODES:(ell + 1) * NUM_NODES].T
    we1ts = np.zeros((P, 48 * FC1), np.float32)
    for k in range(48):
        we1ts[:, k * FC1:(k + 1) * FC1] = we1s[k * P:(k + 1) * P]
    d["we1ts"] = we1ts.astype(BF16)
    d["be1r"] = np.asarray(inp["be1"], np.float32).reshape(1, FC1)
    d["we2t"] = np.ascontiguousarray(
        np.asarray(inp["We2"], np.float32).T.reshape(4, P, FC2)
        .transpose(1, 0, 2).reshape(P, 4 * FC2))
    d["be2r"] = np.asarray(inp["be2"], np.float32).reshape(1, FC2)
    d["we3t"] = np.asarray(inp["We3"], np.float32).T.reshape(P, 1)
    d["be3r"] = np.asarray(inp["be3"], np.float32).reshape(1, 1)
    return d


def kernel(**inputs) -> np.ndarray:
    SPC, shards = _prep_host(inputs["x"], inputs["edge_index"])
    consts = _const_inputs(inputs)
    nc = _get_program(SPC)
    in_maps = []
    for g in range(B):
        m = dict(consts)
        m.update(shards[g])
        in_maps.append(m)
    res = bass_utils.run_bass_kernel_spmd(nc, in_maps, core_ids=list(range(B)))
    pred = np.concatenate([r["pred"].reshape(1, 1) for r in res.results], 0)
    return pred.astype(np.float32)





# revision 24
# speedup vs baseline: 3.1448x; 1.2147x over previous
"""GATv2 (2-layer) + pooling + LayerNorm + MLP encoder — Trainium2 Bass kernel.

Sharding: data-parallel over graphs — 8 graphs, one per NeuronCore. All
conv/pool/FC weights replicated. Inside each core everything runs per-graph
(2000 nodes, 64000 edges + 2000 self-loops).

Node slot space: 16 chunks x 128 slots; real node n -> slot 128*(n//125) +
(n%125), so every chunk has 3 spare slots used as scatter targets for edge
padding. Edges are sorted by destination chunk and padded per chunk to a
uniform SPC slots (128 edges each) so one SPMD program serves all graphs.

Per edge-slot (128 edges across partitions, features along free):
  gather xl[src], xr[dst] rows via gpsimd.dma_gather (DRAM bf16 tables),
  v = xl_s + xr_d; u = leaky_relu(v); t = u * att; logits = reduce_C(t);
  exp -> alpha-numerators; W = xl_s * exp; aggregation + softmax denominator
  via a single PE matmul per slot: onehot(dst)^T @ [W | exp] accumulated in
  PSUM per chunk; epilogue divides by denominator, adds bias, relu.
"""

import sys

sys.path.insert(0, "/opt/trn_rl_repo")

from contextlib import ExitStack

import ml_dtypes
import numpy as np

import concourse.bass as bass
import concourse.bass_isa as bass_isa
import concourse.tile as tile
from concourse import bacc, bass_utils, mybir

BF16 = ml_dtypes.bfloat16
F32 = np.float32
DT = mybir.dt
ALU = mybir.AluOpType
ACT = mybir.ActivationFunctionType

NUM_NODES = 2000
B = 8
IN = 128
NH, C = 4, 32
HC = 128
NEG = 0.2
EPS = 1e-5
FC1, FC2 = 512, 128
P = 128
NCH = 16          # node chunks
CPN = 125         # real nodes per chunk
NPAD = NCH * P    # 2048 slots


def _slot(n):
    return (n // CPN) * P + (n % CPN)


def _ap(a, dims):
    return bass.AP(a.tensor, a.offset, dims)


def _bcast_mid(a, count):
    """[P, F] -> [P, count, F] with step-0 middle dim."""
    return bass.AP(a.tensor, a.offset,
                   [list(a.ap[0]), [0, count], list(a.ap[1])])


def _head_view(a, inner_pitch):
    """[..., 128]-inner AP -> [..., 4, 32] head view."""
    dims = [list(d) for d in a.ap[:-1]] + [[C, NH], [1, C]]
    assert a.ap[-1][1] == HC and a.ap[-1][0] == 1, list(a.ap[-1])
    return bass.AP(a.tensor, a.offset, dims)


def _exp_bcast(a):
    """[..., 4]-inner AP -> [..., 4, 32] with step-0 C dim."""
    dims = [list(d) for d in a.ap[:-1]] + [[1, NH], [0, C]]
    assert a.ap[-1][1] == NH
    return bass.AP(a.tensor, a.offset, dims)


INPUT_SPECS = None  # set below


def _input_specs(SPC):
    NSLOTS = NCH * SPC
    return [
        ("xpad", [NPAD, IN], DT.float32),
        ("ewsrc", [P, NSLOTS * 8], DT.int16),
        ("ewdst", [P, NSLOTS * 8], DT.int16),
        ("dstloc", [P, NSLOTS], DT.bfloat16),
        ("iota", [P, P], DT.bfloat16),
        ("idf", [P, P], DT.float32),
        ("idb", [P, P], DT.bfloat16),
        ("one11", [1, 1], DT.float32),
        ("wl1T", [P, P], DT.float32), ("wr1T", [P, P], DT.float32),
        ("wl2T", [P, P], DT.float32), ("wr2T", [P, P], DT.float32),
        ("bl1b", [P, P], DT.float32), ("br1b", [P, P], DT.float32),
        ("bias1b", [P, P], DT.float32), ("bl2b", [P, P], DT.float32),
        ("br2b", [P, P], DT.float32), ("bias2b", [P, P], DT.float32),
        ("att1b", [P, P], DT.bfloat16), ("att2b", [P, P], DT.bfloat16),
        ("wp1b", [P, P], DT.float32), ("wp2b", [P, P], DT.float32),
        ("lng", [P, NCH], DT.float32),
        ("lnbt", [P, NCH], DT.float32),
        ("maskc", [P, NCH], DT.float32),
        ("we1ts", [P, 48 * FC1], DT.bfloat16),
        ("be1r", [1, FC1], DT.float32),
        ("we2t", [P, 4 * P], DT.float32),
        ("be2r", [1, FC2], DT.float32),
        ("we3t", [P, 1], DT.float32),
        ("be3r", [1, 1], DT.float32),
    ]


def _trace_program(tc, ins, pred_ap, SPC, stop_after=None):
    """ins: dict name -> dram AP; pred_ap: output dram AP."""
    nc = tc.nc
    NSLOTS = NCH * SPC

    class _H:  # minimal handle-like wrapper so .ap() works uniformly
        def __init__(self, ap):
            self._ap = ap

        def ap(self):
            return self._ap

    xpad_d = _H(ins["xpad"])
    ewsrc_d = _H(ins["ewsrc"])
    ewdst_d = _H(ins["ewdst"])
    dstloc_d = _H(ins["dstloc"])
    iota_d = _H(ins["iota"])
    idf_d = _H(ins["idf"])
    idb_d = _H(ins["idb"])
    one11_d = _H(ins["one11"])
    wts = {nm: _H(ins[nm])
           for nm in ["wl1T", "wr1T", "wl2T", "wr2T", "bl1b", "br1b",
                      "bias1b", "bl2b", "br2b", "bias2b", "att1b", "att2b",
                      "wp1b", "wp2b"]}
    lng_d = _H(ins["lng"])
    lnb_d = _H(ins["lnbt"])
    mask_d = _H(ins["maskc"])
    we1_d = _H(ins["we1ts"])
    be1_d = _H(ins["be1r"])
    we2_d = _H(ins["we2t"])
    be2_d = _H(ins["be2r"])
    we3_d = _H(ins["we3t"])
    be3_d = _H(ins["be3r"])
    pred_d = _H(pred_ap)

    # internal DRAM gather tables
    tbl = {nm: nc.dram_tensor(f"{nm}_scr", [NPAD, HC], DT.bfloat16,
                              kind="Internal")
           for nm in ["tl1", "tr1", "tl2", "tr2"]}

    with ExitStack() as ctx:
        pers = ctx.enter_context(tc.tile_pool(name="pers", bufs=1))
        mega = ctx.enter_context(tc.tile_pool(name="mega", bufs=3))
        sm = ctx.enter_context(tc.tile_pool(name="sm", bufs=2))
        psum = ctx.enter_context(tc.tile_pool(name="psum", bufs=2, space="PSUM"))
        psumz = ctx.enter_context(tc.tile_pool(name="psumz", bufs=1, space="PSUM"))

        def load(d, shape, dt, tag):
            t = pers.tile(shape, dt, tag=tag)
            nc.sync.dma_start(t[:], d.ap())
            return t

        # ---- constant loads
        iota_t = load(iota_d, [P, P], DT.bfloat16, "iota")
        idf_t = load(idf_d, [P, P], DT.float32, "idf")
        one11_t = load(one11_d, [1, 1], DT.float32, "one11")
        w = {}
        for nm, d in wts.items():
            dt_w = DT.bfloat16 if nm in ("att1b", "att2b") else DT.float32
            w[nm] = load(d, [P, P], dt_w, nm)
        lng_t = load(lng_d, [P, NCH], DT.float32, "lng")
        lnb_t = load(lnb_d, [P, NCH], DT.float32, "lnbt")
        mask_t = load(mask_d, [P, NCH], DT.float32, "maskc")
        we1_t = load(we1_d, [P, 48 * FC1], DT.bfloat16, "we1")
        be1_t = load(be1_d, [1, FC1], DT.float32, "be1")
        we2_t = load(we2_d, [P, 4 * P], DT.float32, "we2")
        be2_t = load(be2_d, [1, FC2], DT.float32, "be2")
        we3_t = load(we3_d, [P, 1], DT.float32, "we3")
        be3_t = load(be3_d, [1, 1], DT.float32, "be3")
        dstloc_t = load(dstloc_d, [P, NSLOTS], DT.bfloat16, "dstloc")

        # ---- pooled columns
        x0col = pers.tile([P, NCH], DT.float32, tag="x0col")
        x1col = pers.tile([P, NCH], DT.float32, tag="x1col")
        x2col = pers.tile([P, NCH], DT.float32, tag="x2col")

        def make_tables(chunk_src, wl, bl, wr, br, dl, dr, pre=None):
            # per chunk: transpose src chunk on PE, then the two table matmuls
            for c in range(NCH):
                hs = chunk_src(c)
                if pre is not None:
                    pre(c, hs)
                tp = psum.tile([P, P], DT.float32, tag="tp", space="PSUM")
                nc.tensor.transpose(tp[:], hs, idf_t[:])
                hT = sm.tile([P, P], DT.float32, tag="hTc")
                nc.vector.tensor_copy(hT[:], tp[:])
                for wmat, bvec, dst in ((wl, bl, dl), (wr, br, dr)):
                    pm = psum.tile([P, P], DT.float32, tag="tp", space="PSUM")
                    nc.tensor.matmul(pm[:], hT[:], wmat[:],
                                     start=True, stop=True)
                    tb = sm.tile([P, P], DT.bfloat16, tag="tbl")
                    nc.vector.tensor_tensor(tb[:], pm[:], bvec[:], op=ALU.add)
                    nc.sync.dma_start(dst.ap()[c * P:(c + 1) * P, :], tb[:])

        def _finish():
            zf = sm.tile([1, 1], DT.float32, tag="zfin")
            nc.vector.memset(zf[:], 0.0)
            nc.sync.dma_start(pred_d.ap(), zf[:])

        if stop_after == "A0":
            _finish(); return

        def x_chunk(c):
            xc = sm.tile([P, IN], DT.float32, tag="xchunk")
            nc.sync.dma_start(xc[:], xpad_d.ap()[c * P:(c + 1) * P, :])
            return xc[:]

        def x0_pre(c, hs):
            nc.vector.tensor_reduce(x0col[:, c:c + 1], hs,
                                    axis=mybir.AxisListType.X, op=ALU.add)

        make_tables(x_chunk, w["wl1T"], w["bl1b"], w["wr1T"], w["br1b"],
                    tbl["tl1"], tbl["tr1"], pre=x0_pre)
        nc.vector.tensor_scalar(x0col[:], x0col[:], 1.0 / IN, None, op0=ALU.mult)
        if stop_after == "A":
            _finish(); return

        MSL = 32  # slots per gather mega (4096 idx = SWDGE ring limit)

        # packed helper tiles so the big DVE ops hit 2x 16-bit mode:
        # iota_mega[p, sl, d] = d, att_mega[p, sl, f] = att[f]
        iota_mega = pers.tile([P, MSL, P], DT.bfloat16, tag="iotam")
        nc.vector.tensor_copy(
            iota_mega[:],
            bass.AP(iota_t[:].tensor, iota_t[:].offset,
                    [list(iota_t[:].ap[0]), [0, MSL], [1, P]]))
        att_mega = pers.tile([P, MSL, HC], DT.bfloat16, tag="attm")

        def gat_layer(tblL, tblR, att_t, biasO_t, h_out, nch=NCH,
                      skip_gather=False, gather_only=False):
            nslots = nch * SPC
            nmega = (nslots + MSL - 1) // MSL
            wexp_tiles = {}
            agg_tiles = {}
            nc.vector.tensor_copy(att_mega[:], _bcast_mid(att_t[:], MSL))
            for m in range(nmega):
                s0 = m * MSL
                ns = min(MSL, nslots - s0)
                srcg = mega.tile([P, ns, HC], DT.bfloat16, tag="srcg")
                dstg = mega.tile([P, ns, HC], DT.bfloat16, tag="dstg")
                if skip_gather:
                    nc.vector.memset(srcg[:], 0.25)
                    nc.vector.memset(dstg[:], 0.25)
                else:
                    esrc = mega.tile([P, ns * 8], DT.int16, tag="esrc")
                    nc.sync.dma_start(
                        esrc[:], ewsrc_d.ap()[:, s0 * 8:(s0 + ns) * 8])
                    edst = mega.tile([P, ns * 8], DT.int16, tag="edst")
                    nc.sync.dma_start(
                        edst[:], ewdst_d.ap()[:, s0 * 8:(s0 + ns) * 8])
                    # split each gather in two halves on separate SWDGE
                    # queues so ring drains overlap
                    halves = ((0, ns // 2), (ns // 2, ns)) if ns > 1 else ((0, ns),)
                    for gi, (gout, tblg, idxt) in enumerate(
                            ((srcg, tblL, esrc), (dstg, tblR, edst))):
                        for hi, (a, b) in enumerate(halves):
                            nh = b - a
                            nc.gpsimd.dma_gather(
                                gout[:, a:b, :], tblg.ap(),
                                idxt[:, a * 8:b * 8],
                                nh * P, nh * P, elem_size=HC,
                                queue_num=(4 * m + 2 * gi + hi) % 4,
                                single_packet=False)
                if gather_only:
                    continue
                # one-hot megatile: oh[p, sl, d] = (iota[p, d] == dstloc[p, s0+sl])
                oh_mega = mega.tile([P, ns, P], DT.bfloat16, tag="ohm")
                dl = dstloc_t[:, s0:s0 + ns]
                dl_b = bass.AP(dl.tensor, dl.offset,
                               [list(dl.ap[0]), list(dl.ap[1]), [0, P]])
                nc.vector.tensor_tensor(oh_mega[:], iota_mega[:, 0:ns, :],
                                        dl_b, op=ALU.is_equal)
                # edge features computed in-place inside wexp[:, :, 0:HC]
                wexp = mega.tile([P, ns, HC + NH], DT.bfloat16, tag="wexp")
                v = wexp[:, :, 0:HC]
                nc.vector.tensor_tensor(v, srcg[:], dstg[:], op=ALU.add)
                nc.scalar.activation(v, v, ACT.Prelu, alpha=NEG)
                nc.vector.tensor_tensor(v, v, att_mega[:, 0:ns, :],
                                        op=ALU.mult)
                lg = mega.tile([P, ns, NH], DT.float32, tag="lg")
                nc.vector.tensor_reduce(lg[:], _head_view(v, HC + NH),
                                        axis=mybir.AxisListType.X, op=ALU.add)
                nc.scalar.activation(wexp[:, :, HC:HC + NH], lg[:], ACT.Exp)
                nc.vector.tensor_tensor(
                    _head_view(wexp[:, :, 0:HC], HC + NH),
                    _head_view(srcg[:], HC),
                    _exp_bcast(wexp[:, :, HC:HC + NH]),
                    op=ALU.mult)
                wexp_tiles[m] = wexp
                # aggregation for the slots in this mega
                for sl in range(ns):
                    s = s0 + sl
                    c = s // SPC
                    if s == c * SPC:
                        agg = psum.tile([P, HC + NH], DT.float32,
                                        tag="agg", space="PSUM")
                        agg_tiles[c] = agg
                    agg = agg_tiles[c]
                    nc.tensor.matmul(agg[:], oh_mega[:, sl, :], wexp[:, sl, :],
                                     start=(s == c * SPC),
                                     stop=(s == (c + 1) * SPC - 1))
                    if s == (c + 1) * SPC - 1:
                        # epilogue: h = relu(agg / den + bias)
                        den = sm.tile([P, NH], DT.float32, tag="den")
                        nc.vector.tensor_scalar(den[:], agg[:, HC:HC + NH],
                                                1e-6, None, op0=ALU.add)
                        rec = sm.tile([P, NH], DT.float32, tag="rec")
                        nc.vector.reciprocal_approx_fast(rec[:], den[:])
                        t1 = sm.tile([P, P], DT.float32, tag="t1")
                        nc.vector.tensor_tensor(
                            _head_view(t1[:], P),
                            _head_view(agg[:, 0:HC], HC + NH),
                            _exp_bcast(rec[:]), op=ALU.mult)
                        nc.vector.tensor_tensor(t1[:], t1[:], biasO_t[:],
                                                op=ALU.add)
                        nc.scalar.activation(h_out[:, c, :], t1[:], ACT.Relu)
                        del agg_tiles[c]

        if stop_after in ("G1", "G16", "NG1", "NG16", "C1"):
            hx = pers.tile([P, NCH, HC], DT.float32, tag="h1")
            n = 1 if stop_after in ("G1", "NG1", "C1") else NCH
            gat_layer(tbl["tl1"], tbl["tr1"], w["att1b"], w["bias1b"], hx,
                      nch=n, skip_gather=stop_after.startswith("NG"),
                      gather_only=stop_after.startswith("G"))
            _finish(); return

        h1_t = pers.tile([P, NCH, HC], DT.float32, tag="h1")
        gat_layer(tbl["tl1"], tbl["tr1"], w["att1b"], w["bias1b"], h1_t)
        if stop_after == "L1":
            _finish(); return

        # pooling x1, h1 transpose, layer-2 tables
        def pool_into(h_tile, wp_t, outcol):
            for c in range(NCH):
                pt = sm.tile([P, P], DT.float32, tag="ptmp")
                nc.vector.tensor_tensor(pt[:], h_tile[:, c, :], wp_t[:],
                                        op=ALU.mult)
                nc.vector.tensor_reduce(outcol[:, c:c + 1], pt[:],
                                        axis=mybir.AxisListType.X, op=ALU.add)

        pool_into(h1_t, w["wp1b"], x1col)
        make_tables(lambda c: h1_t[:, c, :], w["wl2T"], w["bl2b"],
                    w["wr2T"], w["br2b"], tbl["tl2"], tbl["tr2"])
        if stop_after == "MID":
            _finish(); return

        h2_t = pers.tile([P, NCH, HC], DT.bfloat16, tag="h2")
        gat_layer(tbl["tl2"], tbl["tr2"], w["att2b"], w["bias2b"], h2_t)
        pool_into(h2_t, w["wp2b"], x2col)
        if stop_after == "L2":
            _finish(); return

        # ---- LayerNorm on the three pooled rows -> bf16 columns [P, 48]
        lncols = pers.tile([P, 3 * NCH], DT.float32, tag="lncols")

        def layer_norm(xcol, colbase):
            xm = sm.tile([P, NCH], DT.float32, tag="xm")
            nc.vector.tensor_tensor(xm[:], xcol[:], mask_t[:], op=ALU.mult)
            alr = sm.tile([P, NCH], DT.float32, tag="alr")
            nc.gpsimd.partition_all_reduce(alr[:], xm[:], P,
                                           bass_isa.ReduceOp.add)
            tot = sm.tile([P, 1], DT.float32, tag="tot")
            nc.vector.tensor_reduce(tot[:], alr[:],
                                    axis=mybir.AxisListType.X, op=ALU.add)
            mean = sm.tile([P, 1], DT.float32, tag="mean")
            nc.vector.tensor_scalar(mean[:], tot[:], 1.0 / NUM_NODES, None,
                                    op0=ALU.mult)
            sq = sm.tile([P, NCH], DT.float32, tag="sq")
            nc.vector.tensor_tensor(sq[:], xm[:], xm[:], op=ALU.mult)
            alr2 = sm.tile([P, NCH], DT.float32, tag="alr2")
            nc.gpsimd.partition_all_reduce(alr2[:], sq[:], P,
                                           bass_isa.ReduceOp.add)
            tot2 = sm.tile([P, 1], DT.float32, tag="tot2")
            nc.vector.tensor_reduce(tot2[:], alr2[:],
                                    axis=mybir.AxisListType.X, op=ALU.add)
            msq = sm.tile([P, 1], DT.float32, tag="msq")
            nc.vector.tensor_scalar(msq[:], tot2[:], 1.0 / NUM_NODES, None,
                                    op0=ALU.mult)
            m2 = sm.tile([P, 1], DT.float32, tag="m2")
            nc.vector.tensor_tensor(m2[:], mean[:], mean[:], op=ALU.mult)
            var = sm.tile([P, 1], DT.float32, tag="var")
            nc.vector.tensor_tensor(var[:], msq[:], m2[:], op=ALU.subtract)
            sd = sm.tile([P, 1], DT.float32, tag="sd")
            nc.scalar.activation(sd[:], var[:], ACT.Sqrt, bias=eps_t[:])
            rstd = sm.tile([P, 1], DT.float32, tag="rstd")
            nc.vector.reciprocal_approx_fast(rstd[:], sd[:])
            rg = sm.tile([P, NCH], DT.float32, tag="rg")
            nc.vector.tensor_scalar(rg[:], lng_t[:], rstd[:], None,
                                    op0=ALU.mult)
            lnv = sm.tile([P, NCH], DT.float32, tag="lnv")
            nc.vector.scalar_tensor_tensor(lnv[:], xm[:], mean[:], rg[:],
                                           op0=ALU.subtract, op1=ALU.mult)
            nc.vector.tensor_tensor(lncols[:, colbase:colbase + NCH],
                                    lnv[:], lnb_t[:], op=ALU.add)

        eps_t = pers.tile([P, 1], DT.float32, tag="eps")
        nc.vector.memset(eps_t[:], EPS)

        layer_norm(x0col, 0)
        layer_norm(x1col, NCH)
        layer_norm(x2col, 2 * NCH)
        if stop_after == "LN":
            _finish(); return

        # ---- FC encoder
        lnbf = pers.tile([P, 3 * NCH], DT.bfloat16, tag="lnbf")
        nc.vector.tensor_copy(lnbf[:], lncols[:])
        z1p = psumz.tile([1, FC1], DT.float32, tag="zacc", space="PSUM")
        for k in range(48):
            nc.tensor.matmul(z1p[:], lnbf[:, k:k + 1],
                             we1_t[:, k * FC1:(k + 1) * FC1],
                             start=(k == 0), stop=(k == 47))
        z1s = sm.tile([1, FC1], DT.float32, tag="z1s")
        nc.vector.tensor_tensor(z1s[:], z1p[:], be1_t[:], op=ALU.add)
        z1b = sm.tile([1, FC1], DT.float32, tag="z1b")
        nc.scalar.activation(z1b[:], z1s[:], ACT.Relu)
        z1c = sm.tile([P, 4], DT.float32, tag="z1c")
        for j in range(4):
            tp = psumz.tile([P, 1], DT.float32, tag="tcol", space="PSUM")
            nc.tensor.matmul(tp[:], z1b[:, j * P:(j + 1) * P], one11_t[:],
                             start=True, stop=True)
            nc.vector.tensor_copy(z1c[:, j:j + 1], tp[:])
        z2p = psumz.tile([1, FC2], DT.float32, tag="zacc", space="PSUM")
        for j in range(4):
            nc.tensor.matmul(z2p[:], z1c[:, j:j + 1],
                             we2_t[:, j * P:(j + 1) * P],
                             start=(j == 0), stop=(j == 3))
        z2s = sm.tile([1, FC2], DT.float32, tag="z2s")
        nc.vector.tensor_tensor(z2s[:], z2p[:], be2_t[:], op=ALU.add)
        z2b = sm.tile([1, FC2], DT.float32, tag="z2b")
        nc.scalar.activation(z2b[:], z2s[:], ACT.Relu)
        tp = psumz.tile([P, 1], DT.float32, tag="tcol", space="PSUM")
        nc.tensor.matmul(tp[:], z2b[:], one11_t[:], start=True, stop=True)
        z2c = sm.tile([P, 1], DT.float32, tag="z2c")
        nc.vector.tensor_copy(z2c[:], tp[:])
        z3p = psumz.tile([1, 1], DT.float32, tag="zacc", space="PSUM")
        nc.tensor.matmul(z3p[:], z2c[:], we3_t[:], start=True, stop=True)
        z3s = sm.tile([1, 1], DT.float32, tag="z3s")
        nc.vector.tensor_tensor(z3s[:], z3p[:], be3_t[:], op=ALU.add)
        nc.sync.dma_start(pred_d.ap(), z3s[:])


def _build_program(SPC):
    nc = bacc.Bacc("TRN2", target_bir_lowering=False, debug=False,
                   enable_asserts=False, num_devices=B, num_swdge_queues=4)
    ins = {}
    for nm, shape, dt in _input_specs(SPC):
        ins[nm] = nc.dram_tensor(nm, shape, dt, kind="ExternalInput").ap()
    pred_ap = nc.dram_tensor("pred", [1, 1], DT.float32,
                             kind="ExternalOutput").ap()
    with tile.TileContext(nc) as tc:
        _trace_program(tc, ins, pred_ap, SPC)
    nc.compile()
    return nc


_PROG_CACHE = {}


def _get_program(SPC):
    if SPC not in _PROG_CACHE:
        _PROG_CACHE[SPC] = _build_program(SPC)
    return _PROG_CACHE[SPC]


def _prep_host(x, edge_index):
    """Split into per-graph shards, build slot-space index structures."""
    x = np.asarray(x, dtype=np.float32)
    ei = np.asarray(edge_index)
    src_all = ei[0].astype(np.int64)
    dst_all = ei[1].astype(np.int64)

    graphs = []
    per_chunk_counts = np.zeros((B, NCH), np.int64)
    for g in range(B):
        base = g * NUM_NODES
        m = slice(g * NUM_NODES * 32, (g + 1) * NUM_NODES * 32)
        src = src_all[m] - base
        dst = dst_all[m] - base
        loops = np.arange(NUM_NODES, dtype=np.int64)
        es = _slot(np.concatenate([src, loops]))
        ed = _slot(np.concatenate([dst, loops]))
        order = np.argsort(ed, kind="stable")
        es, ed = es[order], ed[order]
        ch = ed // P
        for c in range(NCH):
            per_chunk_counts[g, c] = int((ch == c).sum())
        graphs.append((es, ed, ch))

    SPC = int(np.ceil(per_chunk_counts.max() / P))
    NSLOTS = NCH * SPC

    shards = []
    for g in range(B):
        es, ed, ch = graphs[g]
        es_p = np.full(NSLOTS * P, 127, np.int64)
        ed_p = np.zeros(NSLOTS * P, np.int64)
        for c in range(NCH):
            sel = ch == c
            cnt = int(sel.sum())
            beg = c * SPC * P
            es_p[beg:beg + cnt] = es[sel]
            ed_p[beg:beg + cnt] = ed[sel]
            ed_p[beg + cnt:(c + 1) * SPC * P] = c * P + 127
        ew_src = np.tile(es_p.reshape(-1, 16).T.astype(np.int16), (8, 1))
        ew_dst = np.tile(ed_p.reshape(-1, 16).T.astype(np.int16), (8, 1))
        dl = (ed_p % P).astype(np.float32)
        dstloc = np.ascontiguousarray(dl.reshape(NSLOTS, P).T).astype(BF16)

        xg = x[g * NUM_NODES:(g + 1) * NUM_NODES]
        xpad = np.zeros((NPAD, IN), np.float32)
        xpad[_slot(np.arange(NUM_NODES))] = xg
        shards.append(dict(xpad=xpad, ewsrc=ew_src, ewdst=ew_dst,
                           dstloc=dstloc))
    return SPC, shards


def _const_inputs(inp):
    """Replicated weight/const arrays keyed by dram tensor name."""
    def bcast_row(vec, dtype=np.float32):
        return np.ascontiguousarray(
            np.broadcast_to(np.asarray(vec, np.float32).reshape(-1), (P, P))
        ).astype(dtype)

    d = {}
    d["iota"] = np.ascontiguousarray(
        np.broadcast_to(np.arange(P, dtype=np.float32), (P, P))).astype(BF16)
    d["idf"] = np.eye(P, dtype=np.float32)
    d["idb"] = np.eye(P, dtype=np.float32).astype(BF16)
    d["one11"] = np.ones((1, 1), np.float32)
    for nm, key in [("wl1T", "Wl1"), ("wr1T", "Wr1"),
                    ("wl2T", "Wl2"), ("wr2T", "Wr2")]:
        d[nm] = np.ascontiguousarray(
            np.asarray(inp[key], np.float32).T)
    for nm, key in [("bl1b", "bl1"), ("br1b", "br1"), ("bias1b", "bias1"),
                    ("bl2b", "bl2"), ("br2b", "br2"), ("bias2b", "bias2")]:
        d[nm] = bcast_row(inp[key])
    d["att1b"] = bcast_row(np.asarray(inp["att1"], np.float32).reshape(-1), BF16)
    d["att2b"] = bcast_row(np.asarray(inp["att2"], np.float32).reshape(-1), BF16)
    d["wp1b"] = bcast_row(np.asarray(inp["Wp1"], np.float32).reshape(-1))
    d["wp2b"] = bcast_row(np.asarray(inp["Wp2"], np.float32).reshape(-1))

    def col_slot(vec, fill=0.0):
        v = np.full(NPAD, fill, np.float32)
        v[_slot(np.arange(NUM_NODES))] = np.asarray(vec, np.float32)
        return np.ascontiguousarray(v.reshape(NCH, P).T)

    d["lng"] = col_slot(inp["ln_g"])
    d["lnbt"] = col_slot(inp["ln_b"])
    d["maskc"] = col_slot(np.ones(NUM_NODES, np.float32))

    # We1 [512, 6000] -> slot space [3*2048, 512] -> [128, 48*512]
    we1 = np.asarray(inp["We1"], np.float32)
    we1s = np.zeros((3 * NPAD, FC1), np.float32)
    sl = _slot(np.arange(NUM_NODES))
    for ell in range(3):
        we1s[ell * NPAD + sl] = we1[:, ell * NUM_NODES:(ell + 1) * NUM_NODES].T
    we1ts = np.zeros((P, 48 * FC1), np.float32)
    for k in range(48):
        we1ts[:, k * FC1:(k + 1) * FC1] = we1s[k * P:(k + 1) * P]
    d["we1ts"] = we1ts.astype(BF16)
    d["be1r"] = np.asarray(inp["be1"], np.float32).reshape(1, FC1)
    d["we2t"] = np.ascontiguousarray(
        np.asarray(inp["We2"], np.float32).T.reshape(4, P, FC2)
        .transpose(1, 0, 2).reshape(P, 4 * FC2))
    d["be2r"] = np.asarray(inp["be2"], np.float32).reshape(1, FC2)
    d["we3t"] = np.asarray(inp["We3"], np.float32).T.reshape(P, 1)
    d["be3r"] = np.asarray(inp["be3"], np.float32).reshape(1, 1)
    return d


def kernel(**inputs) -> np.ndarray:
    SPC, shards = _prep_host(inputs["x"], inputs["edge_index"])
    consts = _const_inputs(inputs)
    nc = _get_program(SPC)
    in_maps = []
    for g in range(B):
        m = dict(consts)
        m.update(shards[g])
        in_maps.append(m)
    res = bass_utils.run_bass_kernel_spmd(nc, in_maps, core_ids=list(range(B)))
    pred = np.concatenate([r["pred"].reshape(1, 1) for r in res.results], 0)
    return pred.astype(np.float32)



